# revision 1
# baseline (speedup 1.0000x reference)
"""Trainium2 Bass kernel for a 2-layer GAT occupancy predictor (B=1).

Reference math:
  pts = concat(pos, pos_non_manifold) -> [K=6000, 3]
  mask[i,j] = ||pts_i - pts_j||^2 < 0.05^2          (dense radius graph)
  layer l:  h = x @ Wl                              [K, 4*64]
            e[i,j,h] = leaky02(ed[i,h] + es[j,h])   es/ed = <h, a_src/dst>
            alpha = softmax_j(e masked)
            x' = relu(alpha @ h + b)
  logits = (x2 @ fc_w + fc_b)[M:] reshaped to [1, 2, 3000]

Distribution (8 NeuronCores): nodes are Morton-sorted on the host so the
radius graph becomes block-local; core c owns destination rows
[768c, 768(c+1)) of the sorted, padded 6144-node graph.  The radius mask is
>99.9% empty, so each core only processes the j-tiles (128 sources) that can
reach its destination block: the host computes each core's active-tile list
(conservative epsilon-superset of the exact mask) and pads every list to a
common NSLOT, so all cores run the same program on different slot data.

Layer 1 is fully static: the host passes the gathered slot points.  Between
layers one AllGather shares the transposed features; layer 2 fetches each
slot's x^T tile from the gathered buffer with indirect DMA driven by a
host-computed index table.

Engine mapping per slot:
  PE   : h (with es riding along as 4 extra host-folded weight columns),
         d2 = |p_i - p_j|^2 as one K=5 matmul ([p; sq; 1] x [-2p; 1; sq]),
         the alpha @ h aggregation as bf16 hi+lo pairs into shared PSUM
         banks, denominators as N=1 ones-column matmuls.
  DVE  : scores e = ed+es (bf16 4x mode), T = 0.2e, leaky = max merged over
         heads (2x mode), part of the mask-adds, h hi/lo split.
  ACT  : one head's leaky via Prelu, exp over all 4 heads in one op,
         PSUM->SBUF copies.
  Pool : the other mask-adds, partition-broadcast of ed.
Mask offsets (-30/0 bf16) stream to DRAM in layer 1 and back in layer 2.
Padded nodes sit at (-1,-1,-1): finite features, outside every real radius.
"""

import sys

sys.path.insert(0, "/opt/trn_rl_repo")

from contextlib import ExitStack

import ml_dtypes
import numpy as np

import concourse.bacc as bacc
import concourse.bass as bass
import concourse.mybir as mybir
import concourse.tile as tile
from concourse.bass_utils import run_bass_kernel_spmd

F32 = mybir.dt.float32
BF16 = mybir.dt.bfloat16
I32 = mybir.dt.int32
AF = mybir.ActivationFunctionType
OP = mybir.AluOpType
AX = mybir.AxisListType

N_CORES = 8
N = 3000
M = 3000
K = N + M          # real nodes
KP = 6144          # padded nodes (48 source tiles of 128)
NT = KP // 128     # 48
IC = KP // N_CORES # 768 destinations per core
ICT = IC // 128    # 6 destination chunks per core
H = 4              # heads
C = 64             # channels per head
HC = H * C         # 256
HCE = HC + H       # h columns + es columns
CP1 = C + 1        # head channels + ones column
R2 = float(np.float32(0.05) * np.float32(0.05))
PAD_COORD = -1.0
MASK_EPS = 1e-5    # host activity-test margin (superset of device mask)
MNEG = -30.0       # masked-score offset: exp(-30+L) ~ 1e-12
PAD_TILE = NT - 1  # tile of all-padding nodes, used for unused slots

# engine for the mask-add per (layer, head): Pool offloads DVE
MN_ENGINE = {(1, 0): "pool", (1, 1): "pool", (1, 2): "dve", (1, 3): "dve",
             (2, 0): "pool", (2, 1): "pool", (2, 2): "dve", (2, 3): "dve"}


def build(nslot, n_cores=N_CORES, fake_ag=False, use_prelu=True):
    nc = bacc.Bacc("TRN2", target_bir_lowering=False, debug=False,
                   num_devices=n_cores)
    NS = nslot

    # ---- kernel I/O (identical program on every core) ----
    # pts_sel5 rows: [p(3); sq; ones] for the slot sources
    # pts_own5 rows: [-2p(3); ones; sq] for the own destination columns
    pts_sel5_d = nc.dram_tensor("pts_sel5", [5, NS * 128], F32,
                                kind="ExternalInput")
    pts_own5_d = nc.dram_tensor("pts_own5", [5, IC], F32,
                                kind="ExternalInput")
    pts_own3_d = nc.dram_tensor("pts_own3", [3, IC], F32,
                                kind="ExternalInput")
    agidx_d = nc.dram_tensor("agidx", [128, NS, 2], I32, kind="ExternalInput")
    # w1p/w2p: [W | W @ a_src_blockdiag] so es rides along with h
    w1p_d = nc.dram_tensor("w1p", [3, HCE], F32, kind="ExternalInput")
    w2p_d = nc.dram_tensor("w2p", [HC, HCE], F32, kind="ExternalInput")
    adm1_d = nc.dram_tensor("adm1", [HC, H], BF16, kind="ExternalInput")
    adm2_d = nc.dram_tensor("adm2", [HC, H], BF16, kind="ExternalInput")
    bias1_d = nc.dram_tensor("bias1", [128, HC], F32, kind="ExternalInput")
    bias2_d = nc.dram_tensor("bias2", [128, HC], F32, kind="ExternalInput")
    fcw_d = nc.dram_tensor("fcw", [128, 2 * HC], F32, kind="ExternalInput")
    fcb_d = nc.dram_tensor("fcb", [128, 2], F32, kind="ExternalInput")
    ident_d = nc.dram_tensor("ident", [128, 128], F32, kind="ExternalInput")

    out_d = nc.dram_tensor("out", [IC, 2], F32, kind="ExternalOutput")

    with tile.TileContext(nc) as tc, ExitStack() as st:
        dram = st.enter_context(tc.tile_pool(name="dram", bufs=1,
                                             space="DRAM"))
        xt_bounce = dram.tile([HC, IC], F32)
        ag_out = dram.tile([n_cores * HC, IC], F32,
                           addr_space=("Local" if fake_ag else "Shared"))
        mn_dram = dram.tile([NS, 128, IC], BF16)   # per-slot mask offsets

        const = st.enter_context(tc.tile_pool(name="const", bufs=1))
        pts_sel5_sb = const.tile([5, NS * 128], F32)
        pts_own5_sb = const.tile([5, IC], F32)
        pts_own3_sb = const.tile([3, IC], F32)
        agidx_sb = const.tile([128, NS, 2], I32)
        w1p_sb = const.tile([3, HCE], F32)
        w2p_sb = const.tile([128, 2, HCE], F32)
        adm1_sb = const.tile([128, 2, H], BF16)
        adm2_sb = const.tile([128, 2, H], BF16)
        bias1_sb = const.tile([128, HC], F32)
        bias2_sb = const.tile([128, HC], F32)
        fcw_sb = const.tile([128, 2 * HC], F32)
        fcb_sb = const.tile([128, 2], F32)
        ident_sb = const.tile([128, 128], F32)

        nc.sync.dma_start(out=pts_sel5_sb[:, :], in_=pts_sel5_d[:, :])
        nc.sync.dma_start(out=pts_own5_sb[:, :], in_=pts_own5_d[:, :])
        nc.sync.dma_start(out=pts_own3_sb[:, :], in_=pts_own3_d[:, :])
        nc.sync.dma_start(out=agidx_sb[:, :, :], in_=agidx_d[:, :, :])
        nc.sync.dma_start(out=w1p_sb[:, :], in_=w1p_d[:, :])
        nc.sync.dma_start(out=w2p_sb[:, :, :],
                          in_=w2p_d.rearrange("(s p) c -> p s c", p=128))
        nc.sync.dma_start(out=adm1_sb[:, :, :],
                          in_=adm1_d.rearrange("(s p) h -> p s h", p=128))
        nc.sync.dma_start(out=adm2_sb[:, :, :],
                          in_=adm2_d.rearrange("(s p) h -> p s h", p=128))
        nc.sync.dma_start(out=bias1_sb[:, :], in_=bias1_d[:, :])
        nc.sync.dma_start(out=bias2_sb[:, :], in_=bias2_d[:, :])
        nc.sync.dma_start(out=fcw_sb[:, :], in_=fcw_d[:, :])
        nc.sync.dma_start(out=fcb_sb[:, :], in_=fcb_d[:, :])
        nc.sync.dma_start(out=ident_sb[:, :], in_=ident_d[:, :])

        big = st.enter_context(tc.tile_pool(name="big", bufs=1))
        hp_hi = big.tile([128, NS, H, CP1], BF16)
        hp_lo = big.tile([128, NS, H, CP1], BF16)
        es4 = big.tile([128, NS, H], F32)
        ed_b = big.tile([128, H, IC], BF16)
        x_sb = big.tile([128, ICT, HC], F32)
        xt_own = big.tile([128, 2, IC], F32)
        edt_sb = big.tile([H, IC], BF16)
        edt_rows = big.tile([1, H, IC], BF16)
        logit_sb = big.tile([128, ICT, 2], F32)

        nc.vector.memset(hp_hi[:, :, :, C:CP1], 1.0)
        nc.vector.memset(hp_lo[:, :, :, C:CP1], 0.0)

        ag_flat = ag_out.rearrange("r (b c) -> (r b) c", c=128)

        # ================= the two GAT layers =================
        for layer in (1, 2):
            adm_sb = adm1_sb if layer == 1 else adm2_sb
            bias_sb = bias1_sb if layer == 1 else bias2_sb

            # ---- own-column side: hT(own), edT, ED broadcasts ----
            with tc.tile_pool(name=f"prep{layer}", bufs=2) as prep, \
                 tc.tile_pool(name=f"prep_ps{layer}", bufs=1,
                              space="PSUM") as prep_ps:
                ht_own = prep.tile([128, 2, IC], BF16, tag="ht", bufs=1)
                for oc in range(2):
                    ht_ps = prep_ps.tile([128, IC], F32, tag="ht_ps", bufs=1,
                                         name=f"ht_ps_{layer}_{oc}")
                    if layer == 1:
                        for lo, sz in ((0, 512), (512, 256)):
                            sl = slice(lo, lo + sz)
                            nc.tensor.matmul(
                                ht_ps[:, sl],
                                w1p_sb[:, oc * 128:(oc + 1) * 128],
                                pts_own3_sb[:, sl], start=True, stop=True)
                    else:
                        for s in range(2):
                            for lo, sz in ((0, 512), (512, 256)):
                                sl = slice(lo, lo + sz)
                                nc.tensor.matmul(
                                    ht_ps[:, sl],
                                    w2p_sb[:, s, oc * 128:(oc + 1) * 128],
                                    xt_own[:, s, sl],
                                    start=(s == 0), stop=(s == 1))
                    nc.scalar.copy(ht_own[:, oc, :], ht_ps[:, :])

                edt_ps = prep_ps.tile([H, IC], F32, tag="edt", bufs=1)
                for s in range(2):
                    for lo, sz in ((0, 512), (512, 256)):
                        sl = slice(lo, lo + sz)
                        nc.tensor.matmul(edt_ps[:, sl], adm_sb[:, s, :],
                                         ht_own[:, s, sl],
                                         start=(s == 0), stop=(s == 1))
                nc.scalar.copy(edt_sb[:, :], edt_ps[:, :])
                for h in range(H):
                    nc.sync.dma_start(out=edt_rows[0:1, h, :],
                                      in_=edt_sb[h:h + 1, :])
                for h in range(H):
                    nc.gpsimd.partition_broadcast(ed_b[:, h, :],
                                                  edt_rows[0:1, h, :])

            # ---- slot loop: h+es, d2 mask, scores, aggregation ----
            with tc.tile_pool(name=f"agg_ps{layer}", bufs=1,
                              space="PSUM") as agg_ps:
                aggp = [agg_ps.tile([128, 2, H, C], F32, tag=f"agg{p}",
                                    name=f"agg_{layer}_{p}")
                        for p in range(ICT // 2)]
                den_ps = agg_ps.tile([128, ICT, H], F32, tag="den",
                                     name=f"den_{layer}")
                with tc.tile_pool(name=f"jl{layer}", bufs=3) as jl, \
                     tc.tile_pool(name=f"h_ps{layer}", bufs=2,
                                  space="PSUM") as h_psp:
                    for s in range(NS):
                        # --- h + es for this slot's 128 sources ---
                        h_ps = h_psp.tile([128, HCE], F32, tag="h",
                                          name=f"h_ps_{layer}_{s}")
                        if layer == 1:
                            nc.tensor.matmul(
                                h_ps[:, :],
                                pts_sel5_sb[0:3, s * 128:(s + 1) * 128],
                                w1p_sb[:, :], start=True, stop=True)
                        else:
                            for half in range(2):
                                xtg = jl.tile([128, 128], F32,
                                              tag=f"xtg{half}",
                                              name=f"xtg_{layer}_{s}_{half}")
                                nc.gpsimd.indirect_dma_start(
                                    out=xtg[:, :], out_offset=None,
                                    in_=ag_flat,
                                    in_offset=bass.IndirectOffsetOnAxis(
                                        ap=agidx_sb[:, s, half:half + 1],
                                        axis=0))
                                nc.tensor.matmul(
                                    h_ps[:, :], xtg[:, :],
                                    w2p_sb[:, half, :],
                                    start=(half == 0), stop=(half == 1))
                        nc.vector.tensor_scalar_add(es4[:, s, :],
                                                    h_ps[:, HC:HCE], 0.0)
                        # h -> bf16 hi + lo with ones/zeros column
                        nc.scalar.copy(
                            hp_hi[:, s, :, 0:C],
                            h_ps[:, 0:HC].rearrange("p (h c) -> p h c", h=H))
                        nc.vector.tensor_tensor(
                            hp_lo[:, s, :, 0:C],
                            h_ps[:, 0:HC].rearrange("p (h c) -> p h c", h=H),
                            hp_hi[:, s, :, 0:C], OP.subtract)

                        # --- mask offsets mn (layer 1: d2 on PE; 2: DRAM) ---
                        mn = jl.tile([128, IC], BF16, tag="mn",
                                     name=f"mn_{layer}_{s}")
                        if layer == 1:
                            for lo, sz in ((0, 512), (512, 256)):
                                sl = slice(lo, lo + sz)
                                g_ps = h_psp.tile([128, sz], F32,
                                                  tag=f"g{lo}", bufs=1,
                                                  name=f"g_{s}_{lo}")
                                nc.tensor.matmul(
                                    g_ps[:, :],
                                    pts_sel5_sb[:, s * 128:(s + 1) * 128],
                                    pts_own5_sb[:, sl],
                                    start=True, stop=True)
                                nc.vector.tensor_scalar(
                                    mn[:, sl], g_ps[:, :], R2, MNEG,
                                    OP.is_ge, OP.mult)
                            nc.sync.dma_start(out=mn_dram[s, :, :],
                                              in_=mn[:, :])
                        else:
                            nc.sync.dma_start(out=mn[:, :],
                                              in_=mn_dram[s, :, :])

                        # --- scores: L = leaky(ed+es) + mn ; A = exp(L) ---
                        L4 = jl.tile([128, H, IC], BF16, tag="L4",
                                     name=f"L4_{layer}_{s}")
                        T4 = jl.tile([128, 3, IC], BF16, tag="T4",
                                     name=f"T4_{layer}_{s}")
                        if use_prelu:
                            nc.scalar.activation(L4[:, 0, :], ed_b[:, 0, :],
                                                 AF.Prelu,
                                                 bias=es4[:, s, 0:1],
                                                 scale=1.0, alpha=0.2)
                        else:
                            T0 = jl.tile([128, IC], BF16, tag="T0",
                                         name=f"T0_{layer}_{s}")
                            nc.vector.tensor_scalar(
                                L4[:, 0, :], ed_b[:, 0, :],
                                es4[:, s, 0:1], None, OP.add)
                            nc.vector.tensor_scalar(
                                T0[:, :], L4[:, 0, :], 0.2, None, OP.mult)
                            nc.vector.tensor_tensor(
                                L4[:, 0, :], L4[:, 0, :], T0[:, :], OP.max)
                        for h in range(1, H):
                            nc.vector.tensor_scalar(
                                L4[:, h, :], ed_b[:, h, :],
                                es4[:, s, h:h + 1], None, OP.add)
                            nc.vector.tensor_scalar(
                                T4[:, h - 1, :], L4[:, h, :], 0.2, None,
                                OP.mult)
                        nc.vector.tensor_tensor(L4[:, 1:4, :], L4[:, 1:4, :],
                                                T4[:, :, :], OP.max)
                        for h in range(H):
                            eng = (nc.gpsimd
                                   if MN_ENGINE[(layer, h)] == "pool"
                                   else nc.vector)
                            eng.tensor_tensor(L4[:, h, :], L4[:, h, :],
                                              mn[:, :], OP.add)
                        A4 = jl.tile([128, H, IC], BF16, tag="A4",
                                     name=f"A4_{layer}_{s}")
                        nc.scalar.activation(A4[:, :, :], L4[:, :, :], AF.Exp)

                        # --- aggregation: hi+lo into one psum group per
                        # bank; two ic-chunks share each bank; denominators
                        # (ones column) accumulate in their own bank ---
                        for h in range(H):
                            for ic in range(ICT):
                                out_ap = aggp[ic // 2][:, ic % 2, h, :]
                                first = (s == 0 and h == 0 and ic % 2 == 0)
                                last = (s == NS - 1 and h == H - 1
                                        and ic % 2 == 1)
                                nc.tensor.matmul(
                                    out_ap,
                                    A4[:, h, ic * 128:(ic + 1) * 128],
                                    hp_hi[:, s, h, 0:C],
                                    start=first, stop=False)
                                nc.tensor.matmul(
                                    out_ap,
                                    A4[:, h, ic * 128:(ic + 1) * 128],
                                    hp_lo[:, s, h, 0:C],
                                    start=False, stop=last)
                                nc.tensor.matmul(
                                    den_ps[:, ic, h:h + 1],
                                    A4[:, h, ic * 128:(ic + 1) * 128],
                                    hp_hi[:, s, h, C:CP1],
                                    start=(s == 0 and h == 0 and ic == 0),
                                    stop=(s == NS - 1 and h == H - 1
                                          and ic == ICT - 1))

                # ---- finalize x = relu(num/den + b); AG or fc ----
                with tc.tile_pool(name=f"fin{layer}", bufs=2) as fin, \
                     tc.tile_pool(name=f"fin_ps{layer}", bufs=2,
                                  space="PSUM") as fin_ps:
                    for ic in range(ICT):
                        rec = fin.tile([128, H], F32, tag="rec",
                                       name=f"rec_{layer}_{ic}")
                        nc.vector.reciprocal(rec[:, :], den_ps[:, ic, :])
                        for h in range(H):
                            nc.vector.scalar_tensor_tensor(
                                x_sb[:, ic, h * C:(h + 1) * C],
                                aggp[ic // 2][:, ic % 2, h, :],
                                rec[:, h:h + 1],
                                bias_sb[:, h * C:(h + 1) * C],
                                OP.mult, OP.add)
                    nc.vector.tensor_scalar(x_sb[:, :, :], x_sb[:, :, :],
                                            0.0, None, OP.max)

                    if layer == 1:
                        for ic in range(ICT):
                            for oc in range(2):
                                t_ps = fin_ps.tile([128, 128], F32,
                                                   tag="t_ps",
                                                   name=f"t_ps_{ic}_{oc}")
                                nc.tensor.transpose(
                                    t_ps[:, :],
                                    x_sb[:, ic, oc * 128:(oc + 1) * 128],
                                    ident_sb[:, :])
                                nc.scalar.copy(
                                    xt_own[:, oc, ic * 128:(ic + 1) * 128],
                                    t_ps[:, :])
                        nc.sync.dma_start(
                            out=xt_bounce.rearrange("(s p) i -> p s i",
                                                    p=128),
                            in_=xt_own[:, :, :])
                        if fake_ag:
                            for r in range(n_cores):
                                nc.sync.dma_start(
                                    out=ag_out[r * HC:(r + 1) * HC, :],
                                    in_=xt_bounce[:, :])
                        else:
                            nc.gpsimd.collective_compute(
                                "AllGather", OP.bypass,
                                replica_groups=[list(range(n_cores))],
                                ins=[xt_bounce.opt()],
                                outs=[ag_out.opt()])
                    else:
                        for ic in range(ICT):
                            for o in range(2):
                                prod = fin.tile([128, HC], F32, tag="prod",
                                                name=f"prod_{ic}_{o}")
                                nc.vector.tensor_tensor(
                                    prod[:, :], x_sb[:, ic, :],
                                    fcw_sb[:, o * HC:(o + 1) * HC], OP.mult)
                                red = fin.tile([128, 1], F32, tag="red",
                                               name=f"red_{ic}_{o}")
                                nc.vector.tensor_reduce(
                                    red[:, :], prod[:, :], AX.X, OP.add)
                                nc.vector.tensor_scalar_add(
                                    logit_sb[:, ic, o:o + 1], red[:, :],
                                    fcb_sb[:, o:o + 1])
                        nc.sync.dma_start(
                            out=out_d.rearrange("(q p) o -> p q o", p=128),
                            in_=logit_sb[:, :, :])

    nc.compile()
    return nc


_BUILD_CACHE = {}


def _get_nc(nslot, use_prelu=True):
    key = (nslot, use_prelu)
    if key not in _BUILD_CACHE:
        _BUILD_CACHE[key] = build(nslot, use_prelu=use_prelu)
    return _BUILD_CACHE[key]


def _morton(p, bits=10):
    q = np.clip((p * (1 << bits)).astype(np.int64), 0, (1 << bits) - 1)
    code = np.zeros(len(p), np.int64)
    for b in range(bits):
        for dim in range(3):
            code |= ((q[:, dim] >> b) & 1) << (3 * b + dim)
    return code


def _plan(pts):
    """Sort nodes spatially, find each core's active source tiles."""
    order = np.argsort(_morton(pts), kind="stable")
    p_sorted = np.full((KP, 3), PAD_COORD, np.float32)
    p_sorted[:K] = pts[order]

    sq = (p_sorted ** 2).sum(-1, dtype=np.float32)
    G = p_sorted @ p_sorted.T
    d2 = sq[None, :] + sq[:, None] - 2.0 * G
    near = d2 < (R2 + MASK_EPS)          # [j, i], conservative superset

    jmaps = []
    for c in range(N_CORES):
        cols = near[:, c * IC:(c + 1) * IC]
        act = cols.reshape(NT, 128, IC).any(axis=(1, 2))
        jmaps.append(np.flatnonzero(act))
    nslot = max(len(j) for j in jmaps)
    jmaps = [np.concatenate([j, np.full(nslot - len(j), PAD_TILE, j.dtype)])
             for j in jmaps]
    return order, p_sorted, jmaps, nslot


def _prep_inputs(pos, pos_non_manifold, W1, a_src1, a_dst1, b1,
                 W2, a_src2, a_dst2, b2, fc_w, fc_b):
    bf16 = ml_dtypes.bfloat16
    pts = np.concatenate([np.asarray(pos, np.float32),
                          np.asarray(pos_non_manifold, np.float32)],
                         axis=2)[0].T  # [K, 3]
    order, p_sorted, jmaps, nslot = _plan(pts)
    sq_sorted = (p_sorted ** 2).sum(-1, dtype=np.float32).astype(np.float32)

    def bcast128(v):
        v = np.asarray(v, np.float32).reshape(-1)
        return np.ascontiguousarray(
            np.broadcast_to(v[None, :], (128, v.size)))

    def blockdiag(a):  # [H, C] -> [HC, H] fp32
        m = np.zeros((HC, H), dtype=np.float32)
        for h in range(H):
            m[h * C:(h + 1) * C, h] = np.asarray(a, np.float32)[h]
        return m

    W1f = np.asarray(W1, np.float32)
    W2f = np.asarray(W2, np.float32)
    w1p = np.concatenate([W1f, W1f @ blockdiag(a_src1)], axis=1)
    w2p = np.concatenate([W2f, W2f @ blockdiag(a_src2)], axis=1)

    shared = {
        "w1p": np.ascontiguousarray(w1p.astype(np.float32)),
        "w2p": np.ascontiguousarray(w2p.astype(np.float32)),
        "adm1": blockdiag(a_dst1).astype(bf16),
        "adm2": blockdiag(a_dst2).astype(bf16),
        "bias1": bcast128(b1),
        "bias2": bcast128(b2),
        "fcw": bcast128(np.asarray(fc_w, np.float32).T),
        "fcb": bcast128(fc_b),
        "ident": np.eye(128, dtype=np.float32),
    }
    in_maps = []
    for c in range(N_CORES):
        jm = jmaps[c]
        sel = (jm[:, None] * 128 + np.arange(128)[None, :]).reshape(-1)
        psel = p_sorted[sel]                      # [nslot*128, 3]
        pown = p_sorted[c * IC:(c + 1) * IC]
        sel5 = np.concatenate(
            [psel.T, sq_sorted[sel][None, :],
             np.ones((1, len(sel)), np.float32)], axis=0)
        own5 = np.concatenate(
            [-2.0 * pown.T, np.ones((1, IC), np.float32),
             (pown ** 2).sum(-1, dtype=np.float32)[None, :]], axis=0)
        r = jm // ICT
        lq = jm % ICT
        agidx = np.zeros((128, nslot, 2), np.int32)
        p_ar = np.arange(128)
        for si in range(nslot):
            for half in range(2):
                rows = r[si] * HC + half * 128 + p_ar
                agidx[:, si, half] = rows * ICT + lq[si]
        m = dict(shared)
        m["pts_sel5"] = np.ascontiguousarray(sel5.astype(np.float32))
        m["pts_own5"] = np.ascontiguousarray(own5.astype(np.float32))
        m["pts_own3"] = np.ascontiguousarray(pown.T)
        m["agidx"] = agidx
        in_maps.append(m)
    return in_maps, order, nslot


def kernel(pos, pos_non_manifold, W1, a_src1, a_dst1, b1,
           W2, a_src2, a_dst2, b2, fc_w, fc_b, _trace=False,
           _use_prelu=True):
    in_maps, order, nslot = _prep_inputs(
        pos, pos_non_manifold, W1, a_src1, a_dst1, b1,
        W2, a_src2, a_dst2, b2, fc_w, fc_b)
    nc = _get_nc(nslot, use_prelu=_use_prelu)
    res = run_bass_kernel_spmd(nc, in_maps, core_ids=list(range(N_CORES)),
                               trace=_trace)
    kernel.last_results = res
    x2s = np.concatenate([res.results[c]["out"] for c in range(N_CORES)],
                         axis=0)  # [KP, 2] in sorted order
    x2 = np.empty((K, 2), np.float32)
    x2[order] = x2s[:K]
    logits = np.ascontiguousarray(x2[M:K]).reshape(1, 2, 3000)
    return logits.astype(np.float32)



# revision 2
# speedup vs baseline: 2.4859x; 2.4859x over previous
"""Trainium2 Bass kernel for a 2-layer GAT occupancy predictor (B=1).

Reference math:
  pts = concat(pos, pos_non_manifold) -> [K=6000, 3]
  mask[i,j] = ||pts_i - pts_j||^2 < 0.05^2          (dense radius graph)
  layer l:  h = x @ Wl                              [K, 4*64]
            e[i,j,h] = leaky02(ed[i,h] + es[j,h])   es/ed = <h, a_src/dst>
            alpha = softmax_j(e masked)
            x' = relu(alpha @ h + b)
  logits = (x2 @ fc_w + fc_b)[M:] reshaped to [1, 2, 3000]

Distribution (8 NeuronCores): nodes are Morton-sorted on the host so the
radius graph becomes block-local; core c owns destination rows
[768c, 768(c+1)) of the sorted, padded 6144-node graph.  The radius mask is
>99.9% empty, so each core only processes the ~870 source nodes that can
reach its destination block: the host gathers them into NSLOT=8 compacted
128-node slots ordered [own 768 | halo | pad], a conservative
epsilon-superset of the exact mask, padded to a common NSLOT so all cores
run the same program on different slot data.

Layer 1 is fully static: the host passes the gathered slot points.  Between
layers one AllGather shares the node-major features x [768, 256]; layer 2
computes own-slot h from the local transposed features and fetches the halo
slots' x rows from the gathered buffer with one indirect DMA per slot,
transposing on the PE.

Engine mapping per slot:
  PE   : h (with es riding along as 4 extra host-folded weight columns),
         d2 = |p_i - p_j|^2 as one K=5 matmul ([p; sq; 1] x [-2p; 1; sq]),
         the alpha @ h aggregation as bf16 hi+lo pairs into shared PSUM
         banks, denominators as N=1 ones-column matmuls.
  DVE  : scores e = ed+es, T = 0.2e, leaky = max merged over
         heads (2x mode), part of the mask-adds, h hi/lo split.
  ACT  : one head's leaky via Prelu, exp over all 4 heads in one op,
         PSUM->SBUF copies.
  Pool : the other mask-adds, partition-broadcast of ed.
Mask offsets (-30/0 bf16) stay resident in SBUF across both layers.
Padded nodes sit at (-1,-1,-1): finite features, outside every real radius.
"""

import sys

sys.path.insert(0, "/opt/trn_rl_repo")

from contextlib import ExitStack

import ml_dtypes
import numpy as np

import concourse.bacc as bacc
import concourse.bass as bass
import concourse.mybir as mybir
import concourse.tile as tile
from concourse.bass_utils import run_bass_kernel_spmd

F32 = mybir.dt.float32
BF16 = mybir.dt.bfloat16
I32 = mybir.dt.int32
AF = mybir.ActivationFunctionType
OP = mybir.AluOpType
AX = mybir.AxisListType

N_CORES = 8
N = 3000
M = 3000
K = N + M          # real nodes
KP = 6144          # padded nodes
NT = KP // 128     # 48
IC = KP // N_CORES # 768 destinations per core
ICT = IC // 128    # 6 destination chunks per core
OWN_SLOTS = ICT    # first 6 slots are the core's own nodes
H = 4              # heads
C = 64             # channels per head
HC = H * C         # 256
HCE = HC + H       # h columns + es columns
CP1 = C + 1        # head channels + ones column
R2 = float(np.float32(0.05) * np.float32(0.05))
PAD_COORD = -1.0
MASK_EPS = 1e-5    # host activity-test margin (superset of device mask)
MNEG = -30.0       # masked-score offset: exp(-30+L) ~ 1e-12
PAD_NODE = KP - 1  # all-padding node, used for unused slot entries

# engine for the mask-add per (layer, head): Pool offloads DVE
MN_ENGINE = {(1, 0): "pool", (1, 1): "pool", (1, 2): "dve", (1, 3): "dve",
             (2, 0): "pool", (2, 1): "pool", (2, 2): "dve", (2, 3): "dve"}


def build(nslot, n_cores=N_CORES, fake_ag=False, use_prelu=True):
    nc = bacc.Bacc("TRN2", target_bir_lowering=False, debug=False,
                   num_devices=n_cores)
    NS = nslot
    NH = NS - OWN_SLOTS          # halo slots (gathered in layer 2)
    assert NH >= 1

    # ---- kernel I/O (identical program on every core) ----
    # pts_sel5 rows: [p(3); sq; ones] for the slot sources
    # pts_own5 rows: [-2p(3); ones; sq] for the own destination columns
    pts_sel5_d = nc.dram_tensor("pts_sel5", [5, NS * 128], F32,
                                kind="ExternalInput")
    pts_own5_d = nc.dram_tensor("pts_own5", [5, IC], F32,
                                kind="ExternalInput")
    pts_own3_d = nc.dram_tensor("pts_own3", [3, IC], F32,
                                kind="ExternalInput")
    agidx_d = nc.dram_tensor("agidx", [128, NH], I32, kind="ExternalInput")
    # w1p/w2p: [W | W @ a_src_blockdiag] so es rides along with h
    w1p_d = nc.dram_tensor("w1p", [3, HCE], F32, kind="ExternalInput")
    w2p_d = nc.dram_tensor("w2p", [HC, HCE], F32, kind="ExternalInput")
    adm1_d = nc.dram_tensor("adm1", [HC, H], BF16, kind="ExternalInput")
    adm2_d = nc.dram_tensor("adm2", [HC, H], BF16, kind="ExternalInput")
    bias1_d = nc.dram_tensor("bias1", [128, HC], F32, kind="ExternalInput")
    bias2_d = nc.dram_tensor("bias2", [128, HC], F32, kind="ExternalInput")
    fcw_d = nc.dram_tensor("fcw", [128, 2 * HC], F32, kind="ExternalInput")
    fcb_d = nc.dram_tensor("fcb", [128, 2], F32, kind="ExternalInput")
    ident_d = nc.dram_tensor("ident", [128, 128], F32, kind="ExternalInput")

    out_d = nc.dram_tensor("out", [IC, 2], F32, kind="ExternalOutput")

    with tile.TileContext(nc) as tc, ExitStack() as st:
        dram = st.enter_context(tc.tile_pool(name="dram", bufs=1,
                                             space="DRAM"))
        x_bounce = dram.tile([IC, HC], F32)
        ag_out = dram.tile([KP, HC], F32,
                           addr_space=("Local" if fake_ag else "Shared"))

        const = st.enter_context(tc.tile_pool(name="const", bufs=1))
        pts_sel5_sb = const.tile([5, NS * 128], F32)
        pts_own5_sb = const.tile([5, IC], F32)
        pts_own3_sb = const.tile([3, IC], F32)
        agidx_sb = const.tile([128, NH], I32)
        w1p_sb = const.tile([3, HCE], F32)
        w2p_sb = const.tile([128, 2, HCE], F32)
        adm1_sb = const.tile([128, 2, H], BF16)
        adm2_sb = const.tile([128, 2, H], BF16)
        bias1_sb = const.tile([128, HC], F32)
        bias2_sb = const.tile([128, HC], F32)
        fcw_sb = const.tile([128, 2 * HC], F32)
        fcb_sb = const.tile([128, 2], F32)
        ident_sb = const.tile([128, 128], F32)

        nc.sync.dma_start(out=pts_sel5_sb[:, :], in_=pts_sel5_d[:, :])
        nc.sync.dma_start(out=pts_own5_sb[:, :], in_=pts_own5_d[:, :])
        nc.sync.dma_start(out=pts_own3_sb[:, :], in_=pts_own3_d[:, :])
        nc.sync.dma_start(out=agidx_sb[:, :], in_=agidx_d[:, :])
        nc.sync.dma_start(out=w1p_sb[:, :], in_=w1p_d[:, :])
        nc.sync.dma_start(out=w2p_sb[:, :, :],
                          in_=w2p_d.rearrange("(s p) c -> p s c", p=128))
        nc.sync.dma_start(out=adm1_sb[:, :, :],
                          in_=adm1_d.rearrange("(s p) h -> p s h", p=128))
        nc.sync.dma_start(out=adm2_sb[:, :, :],
                          in_=adm2_d.rearrange("(s p) h -> p s h", p=128))
        nc.sync.dma_start(out=bias1_sb[:, :], in_=bias1_d[:, :])
        nc.sync.dma_start(out=bias2_sb[:, :], in_=bias2_d[:, :])
        nc.sync.dma_start(out=fcw_sb[:, :], in_=fcw_d[:, :])
        nc.sync.dma_start(out=fcb_sb[:, :], in_=fcb_d[:, :])
        nc.sync.dma_start(out=ident_sb[:, :], in_=ident_d[:, :])

        big = st.enter_context(tc.tile_pool(name="big", bufs=1))
        hp_hi = big.tile([128, NS, H, CP1], BF16)
        hp_lo = big.tile([128, NS, H, CP1], BF16)
        es4 = big.tile([128, NS, H], F32)
        ed_b = big.tile([128, H, IC], BF16)
        x_sb = big.tile([128, ICT, HC], F32)
        xt_own = big.tile([128, 2, IC], F32)
        edt_sb = big.tile([H, IC], BF16)
        edt_rows = big.tile([1, H, IC], BF16)
        logit_sb = big.tile([128, ICT, 2], F32)
        mn_sb = big.tile([128, NS, IC], BF16)   # per-slot mask offsets

        nc.vector.memset(hp_hi[:, :, :, C:CP1], 1.0)
        nc.vector.memset(hp_lo[:, :, :, C:CP1], 0.0)

        # ================= the two GAT layers =================
        for layer in (1, 2):
            adm_sb = adm1_sb if layer == 1 else adm2_sb
            bias_sb = bias1_sb if layer == 1 else bias2_sb

            # ---- own-column side: hT(own), edT, ED broadcasts ----
            with tc.tile_pool(name=f"prep{layer}", bufs=2) as prep, \
                 tc.tile_pool(name=f"prep_ps{layer}", bufs=1,
                              space="PSUM") as prep_ps:
                ht_own = prep.tile([128, 2, IC], BF16, tag="ht", bufs=1)
                for oc in range(2):
                    ht_ps = prep_ps.tile([128, IC], F32, tag="ht_ps", bufs=1,
                                         name=f"ht_ps_{layer}_{oc}")
                    if layer == 1:
                        for lo, sz in ((0, 512), (512, 256)):
                            sl = slice(lo, lo + sz)
                            nc.tensor.matmul(
                                ht_ps[:, sl],
                                w1p_sb[:, oc * 128:(oc + 1) * 128],
                                pts_own3_sb[:, sl], start=True, stop=True)
                    else:
                        for s in range(2):
                            for lo, sz in ((0, 512), (512, 256)):
                                sl = slice(lo, lo + sz)
                                nc.tensor.matmul(
                                    ht_ps[:, sl],
                                    w2p_sb[:, s, oc * 128:(oc + 1) * 128],
                                    xt_own[:, s, sl],
                                    start=(s == 0), stop=(s == 1))
                    nc.scalar.copy(ht_own[:, oc, :], ht_ps[:, :])

                edt_ps = prep_ps.tile([H, IC], F32, tag="edt", bufs=1)
                for s in range(2):
                    for lo, sz in ((0, 512), (512, 256)):
                        sl = slice(lo, lo + sz)
                        nc.tensor.matmul(edt_ps[:, sl], adm_sb[:, s, :],
                                         ht_own[:, s, sl],
                                         start=(s == 0), stop=(s == 1))
                nc.scalar.copy(edt_sb[:, :], edt_ps[:, :])
                for h in range(H):
                    nc.sync.dma_start(out=edt_rows[0:1, h, :],
                                      in_=edt_sb[h:h + 1, :])
                for h in range(H):
                    nc.gpsimd.partition_broadcast(ed_b[:, h, :],
                                                  edt_rows[0:1, h, :])

            # ---- slot loop: h+es, d2 mask, scores, aggregation ----
            with tc.tile_pool(name=f"agg_ps{layer}", bufs=1,
                              space="PSUM") as agg_ps:
                aggp = [agg_ps.tile([128, 2, H, C], F32, tag=f"agg{p}",
                                    name=f"agg_{layer}_{p}")
                        for p in range(ICT // 2)]
                den_ps = agg_ps.tile([128, ICT, H], F32, tag="den",
                                     name=f"den_{layer}")
                with tc.tile_pool(name=f"jl{layer}", bufs=3) as jl, \
                     tc.tile_pool(name=f"h_ps{layer}", bufs=2,
                                  space="PSUM") as h_psp:
                    for s in range(NS):
                        # --- h + es for this slot's 128 sources ---
                        h_ps = h_psp.tile([128, HCE], F32, tag="h",
                                          name=f"h_ps_{layer}_{s}")
                        if layer == 1:
                            nc.tensor.matmul(
                                h_ps[:, :],
                                pts_sel5_sb[0:3, s * 128:(s + 1) * 128],
                                w1p_sb[:, :], start=True, stop=True)
                        elif s < OWN_SLOTS:
                            for half in range(2):
                                nc.tensor.matmul(
                                    h_ps[:, :],
                                    xt_own[:, half,
                                           s * 128:(s + 1) * 128],
                                    w2p_sb[:, half, :],
                                    start=(half == 0), stop=(half == 1))
                        else:
                            xg = jl.tile([128, HC], F32, tag="xg",
                                         name=f"xg_{layer}_{s}")
                            nc.gpsimd.indirect_dma_start(
                                out=xg[:, :], out_offset=None,
                                in_=ag_out,
                                in_offset=bass.IndirectOffsetOnAxis(
                                    ap=agidx_sb[:, s - OWN_SLOTS:
                                                s - OWN_SLOTS + 1],
                                    axis=0))
                            xtg = jl.tile([128, 2, 128], F32, tag="xtg",
                                          name=f"xtg_{layer}_{s}")
                            for half in range(2):
                                t_ps = h_psp.tile(
                                    [128, 128], F32, tag=f"tr{half}",
                                    bufs=1, name=f"tr_{layer}_{s}_{half}")
                                nc.tensor.transpose(
                                    t_ps[:, :],
                                    xg[:, half * 128:(half + 1) * 128],
                                    ident_sb[:, :])
                                nc.scalar.copy(xtg[:, half, :], t_ps[:, :])
                            for half in range(2):
                                nc.tensor.matmul(
                                    h_ps[:, :], xtg[:, half, :],
                                    w2p_sb[:, half, :],
                                    start=(half == 0), stop=(half == 1))
                        nc.vector.tensor_scalar_add(es4[:, s, :],
                                                    h_ps[:, HC:HCE], 0.0)
                        # h -> bf16 hi + lo with ones/zeros column
                        nc.scalar.copy(
                            hp_hi[:, s, :, 0:C],
                            h_ps[:, 0:HC].rearrange("p (h c) -> p h c", h=H))
                        nc.vector.tensor_tensor(
                            hp_lo[:, s, :, 0:C],
                            h_ps[:, 0:HC].rearrange("p (h c) -> p h c", h=H),
                            hp_hi[:, s, :, 0:C], OP.subtract)

                        # --- mask offsets mn (layer 1: d2 on PE) ---
                        if layer == 1:
                            for lo, sz in ((0, 512), (512, 256)):
                                sl = slice(lo, lo + sz)
                                g_ps = h_psp.tile([128, sz], F32,
                                                  tag=f"g{lo}", bufs=1,
                                                  name=f"g_{s}_{lo}")
                                nc.tensor.matmul(
                                    g_ps[:, :],
                                    pts_sel5_sb[:, s * 128:(s + 1) * 128],
                                    pts_own5_sb[:, sl],
                                    start=True, stop=True)
                                nc.vector.tensor_scalar(
                                    mn_sb[:, s, sl], g_ps[:, :], R2, MNEG,
                                    OP.is_ge, OP.mult)

                        # --- scores: L = leaky(ed+es) + mn ; A = exp(L) ---
                        L4 = jl.tile([128, H, IC], BF16, tag="L4",
                                     name=f"L4_{layer}_{s}")
                        T4 = jl.tile([128, 3, IC], BF16, tag="T4",
                                     name=f"T4_{layer}_{s}")
                        if use_prelu:
                            nc.scalar.activation(L4[:, 0, :], ed_b[:, 0, :],
                                                 AF.Prelu,
                                                 bias=es4[:, s, 0:1],
                                                 scale=1.0, alpha=0.2)
                        else:
                            T0 = jl.tile([128, IC], BF16, tag="T0",
                                         name=f"T0_{layer}_{s}")
                            nc.vector.tensor_scalar(
                                L4[:, 0, :], ed_b[:, 0, :],
                                es4[:, s, 0:1], None, OP.add)
                            nc.vector.tensor_scalar(
                                T0[:, :], L4[:, 0, :], 0.2, None, OP.mult)
                            nc.vector.tensor_tensor(
                                L4[:, 0, :], L4[:, 0, :], T0[:, :], OP.max)
                        for h in range(1, H):
                            nc.vector.tensor_scalar(
                                L4[:, h, :], ed_b[:, h, :],
                                es4[:, s, h:h + 1], None, OP.add)
                            nc.vector.tensor_scalar(
                                T4[:, h - 1, :], L4[:, h, :], 0.2, None,
                                OP.mult)
                        nc.vector.tensor_tensor(L4[:, 1:4, :], L4[:, 1:4, :],
                                                T4[:, :, :], OP.max)
                        for h in range(H):
                            eng = (nc.gpsimd
                                   if MN_ENGINE[(layer, h)] == "pool"
                                   else nc.vector)
                            eng.tensor_tensor(L4[:, h, :], L4[:, h, :],
                                              mn_sb[:, s, :], OP.add)
                        A4 = jl.tile([128, H, IC], BF16, tag="A4",
                                     name=f"A4_{layer}_{s}")
                        nc.scalar.activation(A4[:, :, :], L4[:, :, :], AF.Exp)

                        # --- aggregation: hi+lo into one psum group per
                        # bank; two ic-chunks share each bank; denominators
                        # (ones column) accumulate in their own bank ---
                        for h in range(H):
                            for ic in range(ICT):
                                out_ap = aggp[ic // 2][:, ic % 2, h, :]
                                first = (s == 0 and h == 0 and ic % 2 == 0)
                                last = (s == NS - 1 and h == H - 1
                                        and ic % 2 == 1)
                                nc.tensor.matmul(
                                    out_ap,
                                    A4[:, h, ic * 128:(ic + 1) * 128],
                                    hp_hi[:, s, h, 0:C],
                                    start=first, stop=False)
                                nc.tensor.matmul(
                                    out_ap,
                                    A4[:, h, ic * 128:(ic + 1) * 128],
                                    hp_lo[:, s, h, 0:C],
                                    start=False, stop=last)
                                nc.tensor.matmul(
                                    den_ps[:, ic, h:h + 1],
                                    A4[:, h, ic * 128:(ic + 1) * 128],
                                    hp_hi[:, s, h, C:CP1],
                                    start=(s == 0 and h == 0 and ic == 0),
                                    stop=(s == NS - 1 and h == H - 1
                                          and ic == ICT - 1))

                # ---- finalize x = relu(num/den + b); AG or fc ----
                with tc.tile_pool(name=f"fin{layer}", bufs=2) as fin, \
                     tc.tile_pool(name=f"fin_ps{layer}", bufs=2,
                                  space="PSUM") as fin_ps:
                    for ic in range(ICT):
                        rec = fin.tile([128, H], F32, tag="rec",
                                       name=f"rec_{layer}_{ic}")
                        nc.vector.reciprocal(rec[:, :], den_ps[:, ic, :])
                        for h in range(H):
                            nc.vector.scalar_tensor_tensor(
                                x_sb[:, ic, h * C:(h + 1) * C],
                                aggp[ic // 2][:, ic % 2, h, :],
                                rec[:, h:h + 1],
                                bias_sb[:, h * C:(h + 1) * C],
                                OP.mult, OP.add)
                    nc.vector.tensor_scalar(x_sb[:, :, :], x_sb[:, :, :],
                                            0.0, None, OP.max)

                    if layer == 1:
                        # node-major payload: AG starts as soon as x is done
                        nc.sync.dma_start(
                            out=x_bounce.rearrange("(q p) c -> p q c",
                                                   p=128),
                            in_=x_sb[:, :, :])
                        if fake_ag:
                            for r in range(n_cores):
                                nc.sync.dma_start(
                                    out=ag_out[r * IC:(r + 1) * IC, :],
                                    in_=x_bounce[:, :])
                        else:
                            nc.gpsimd.collective_compute(
                                "AllGather", OP.bypass,
                                replica_groups=[list(range(n_cores))],
                                ins=[x_bounce.opt()],
                                outs=[ag_out.opt()])
                        # transposed own x, needed for layer-2 prep
                        for ic in range(ICT):
                            for oc in range(2):
                                t_ps = fin_ps.tile([128, 128], F32,
                                                   tag="t_ps",
                                                   name=f"t_ps_{ic}_{oc}")
                                nc.tensor.transpose(
                                    t_ps[:, :],
                                    x_sb[:, ic, oc * 128:(oc + 1) * 128],
                                    ident_sb[:, :])
                                nc.scalar.copy(
                                    xt_own[:, oc, ic * 128:(ic + 1) * 128],
                                    t_ps[:, :])
                    else:
                        for ic in range(ICT):
                            for o in range(2):
                                prod = fin.tile([128, HC], F32, tag="prod",
                                                name=f"prod_{ic}_{o}")
                                nc.vector.tensor_tensor(
                                    prod[:, :], x_sb[:, ic, :],
                                    fcw_sb[:, o * HC:(o + 1) * HC], OP.mult)
                                red = fin.tile([128, 1], F32, tag="red",
                                               name=f"red_{ic}_{o}")
                                nc.vector.tensor_reduce(
                                    red[:, :], prod[:, :], AX.X, OP.add)
                                nc.vector.tensor_scalar_add(
                                    logit_sb[:, ic, o:o + 1], red[:, :],
                                    fcb_sb[:, o:o + 1])
                        nc.sync.dma_start(
                            out=out_d.rearrange("(q p) o -> p q o", p=128),
                            in_=logit_sb[:, :, :])

    nc.compile()
    return nc


_BUILD_CACHE = {}


def _get_nc(nslot, use_prelu=True):
    key = (nslot, use_prelu)
    if key not in _BUILD_CACHE:
        _BUILD_CACHE[key] = build(nslot, use_prelu=use_prelu)
    return _BUILD_CACHE[key]


def _morton(p, bits=10):
    q = np.clip((p * (1 << bits)).astype(np.int64), 0, (1 << bits) - 1)
    code = np.zeros(len(p), np.int64)
    for b in range(bits):
        for dim in range(3):
            code |= ((q[:, dim] >> b) & 1) << (3 * b + dim)
    return code


def _plan(pts):
    """Sort nodes spatially, build each core's compacted source list."""
    order = np.argsort(_morton(pts), kind="stable")
    p_sorted = np.full((KP, 3), PAD_COORD, np.float32)
    p_sorted[:K] = pts[order]

    sq = (p_sorted ** 2).sum(-1, dtype=np.float32)
    G = p_sorted @ p_sorted.T
    d2 = sq[None, :] + sq[:, None] - 2.0 * G
    near = d2 < (R2 + MASK_EPS)          # [j, i], conservative superset

    lists = []
    for c in range(N_CORES):
        act = np.flatnonzero(near[:, c * IC:(c + 1) * IC].any(axis=1))
        own = np.arange(c * IC, (c + 1) * IC)
        halo = act[(act < c * IC) | (act >= (c + 1) * IC)]
        lists.append(np.concatenate([own, halo]))
    nslot = max(OWN_SLOTS + 1,
                max((len(l) + 127) // 128 for l in lists))
    lists = [np.concatenate(
        [l, np.full(nslot * 128 - len(l), PAD_NODE, l.dtype)])
        for l in lists]
    return order, p_sorted, lists, nslot


def _prep_inputs(pos, pos_non_manifold, W1, a_src1, a_dst1, b1,
                 W2, a_src2, a_dst2, b2, fc_w, fc_b):
    bf16 = ml_dtypes.bfloat16
    pts = np.concatenate([np.asarray(pos, np.float32),
                          np.asarray(pos_non_manifold, np.float32)],
                         axis=2)[0].T  # [K, 3]
    order, p_sorted, lists, nslot = _plan(pts)
    sq_sorted = (p_sorted ** 2).sum(-1, dtype=np.float32).astype(np.float32)

    def bcast128(v):
        v = np.asarray(v, np.float32).reshape(-1)
        return np.ascontiguousarray(
            np.broadcast_to(v[None, :], (128, v.size)))

    def blockdiag(a):  # [H, C] -> [HC, H] fp32
        m = np.zeros((HC, H), dtype=np.float32)
        for h in range(H):
            m[h * C:(h + 1) * C, h] = np.asarray(a, np.float32)[h]
        return m

    W1f = np.asarray(W1, np.float32)
    W2f = np.asarray(W2, np.float32)
    w1p = np.concatenate([W1f, W1f @ blockdiag(a_src1)], axis=1)
    w2p = np.concatenate([W2f, W2f @ blockdiag(a_src2)], axis=1)

    shared = {
        "w1p": np.ascontiguousarray(w1p.astype(np.float32)),
        "w2p": np.ascontiguousarray(w2p.astype(np.float32)),
        "adm1": blockdiag(a_dst1).astype(bf16),
        "adm2": blockdiag(a_dst2).astype(bf16),
        "bias1": bcast128(b1),
        "bias2": bcast128(b2),
        "fcw": bcast128(np.asarray(fc_w, np.float32).T),
        "fcb": bcast128(fc_b),
        "ident": np.eye(128, dtype=np.float32),
    }
    in_maps = []
    for c in range(N_CORES):
        sel = lists[c]
        psel = p_sorted[sel]                      # [nslot*128, 3]
        pown = p_sorted[c * IC:(c + 1) * IC]
        sel5 = np.concatenate(
            [psel.T, sq_sorted[sel][None, :],
             np.ones((1, len(sel)), np.float32)], axis=0)
        own5 = np.concatenate(
            [-2.0 * pown.T, np.ones((1, IC), np.float32),
             (pown ** 2).sum(-1, dtype=np.float32)[None, :]], axis=0)
        # halo slots index node-major ag rows directly
        agidx = np.ascontiguousarray(
            sel[OWN_SLOTS * 128:].reshape(-1, 128).T.astype(np.int32))
        m = dict(shared)
        m["pts_sel5"] = np.ascontiguousarray(sel5.astype(np.float32))
        m["pts_own5"] = np.ascontiguousarray(own5.astype(np.float32))
        m["pts_own3"] = np.ascontiguousarray(pown.T)
        m["agidx"] = agidx
        in_maps.append(m)
    return in_maps, order, nslot


def kernel(pos, pos_non_manifold, W1, a_src1, a_dst1, b1,
           W2, a_src2, a_dst2, b2, fc_w, fc_b, _trace=False,
           _use_prelu=True):
    in_maps, order, nslot = _prep_inputs(
        pos, pos_non_manifold, W1, a_src1, a_dst1, b1,
        W2, a_src2, a_dst2, b2, fc_w, fc_b)
    nc = _get_nc(nslot, use_prelu=_use_prelu)
    res = run_bass_kernel_spmd(nc, in_maps, core_ids=list(range(N_CORES)),
                               trace=_trace)
    kernel.last_results = res
    x2s = np.concatenate([res.results[c]["out"] for c in range(N_CORES)],
                         axis=0)  # [KP, 2] in sorted order
    x2 = np.empty((K, 2), np.float32)
    x2[order] = x2s[:K]
    logits = np.ascontiguousarray(x2[M:K]).reshape(1, 2, 3000)
    return logits.astype(np.float32)


# revision 15
# speedup vs baseline: 2.7642x; 1.1120x over previous
"""Trainium2 Bass kernel for a 2-layer GAT occupancy predictor (B=1).

Reference math:
  pts = concat(pos, pos_non_manifold) -> [K=6000, 3]
  mask[i,j] = ||pts_i - pts_j||^2 < 0.05^2          (dense radius graph)
  layer l:  h = x @ Wl                              [K, 4*64]
            e[i,j,h] = leaky02(ed[i,h] + es[j,h])   es/ed = <h, a_src/dst>
            alpha = softmax_j(e masked)
            x' = relu(alpha @ h + b)
  logits = (x2 @ fc_w + fc_b)[M:] reshaped to [1, 2, 3000]

Distribution (8 NeuronCores): nodes are Morton-sorted on the host so the
radius graph becomes block-local; core c owns destination rows
[768c, 768(c+1)) of the sorted, padded 6144-node graph.  The radius mask is
>99.9% empty, so each core only processes the ~870 source nodes that can
reach its destination block: the host gathers them into NSLOT=8 compacted
128-node slots ordered [own 768 | halo | pad], a conservative
epsilon-superset of the exact mask, padded to a common NSLOT so all cores
run the same program on different slot data.

Layer 1 is fully static: the host passes the gathered slot points.  Between
layers one AllGather shares the node-major features x [768, 256]; layer 2
computes own-slot h from the local transposed features and fetches the halo
slots' x rows from the gathered buffer with one indirect DMA per slot,
transposing on the PE.

Engine mapping per slot:
  PE   : h (with es riding along as 4 extra host-folded weight columns),
         d2 = |p_i - p_j|^2 as one K=5 matmul ([p; sq; 1] x [-2p; 1; sq]),
         the alpha @ h aggregation as bf16 hi+lo pairs into shared PSUM
         banks, denominators as N=1 ones-column matmuls.
  DVE  : scores e = ed+es, T = 0.2e, leaky = max merged over
         heads (2x mode), part of the mask-adds, h hi/lo split.
  ACT  : one head's leaky via Prelu, exp over all 4 heads in one op,
         PSUM->SBUF copies.
  Pool : the other mask-adds, partition-broadcast of ed.
Mask offsets (-30/0 bf16) stay resident in SBUF across both layers.
Padded nodes sit at (-1,-1,-1): finite features, outside every real radius.
"""

import sys

sys.path.insert(0, "/opt/trn_rl_repo")

from contextlib import ExitStack

import ml_dtypes
import numpy as np

import concourse.bacc as bacc
import concourse.bass as bass
import concourse.mybir as mybir
import concourse.tile as tile
from concourse.bass_utils import run_bass_kernel_spmd

F32 = mybir.dt.float32
BF16 = mybir.dt.bfloat16
I32 = mybir.dt.int32
AF = mybir.ActivationFunctionType
OP = mybir.AluOpType
AX = mybir.AxisListType

N_CORES = 8
N = 3000
M = 3000
K = N + M          # real nodes
KP = 6144          # padded nodes
NT = KP // 128     # 48
IC = KP // N_CORES # 768 destinations per core
ICT = IC // 128    # 6 destination chunks per core
OWN_SLOTS = ICT    # first 6 slots are the core's own nodes
H = 4              # heads
C = 64             # channels per head
HC = H * C         # 256
HCE = HC + H       # h columns + es columns
CP1 = C + 1        # head channels + ones column
R2 = float(np.float32(0.05) * np.float32(0.05))
PAD_COORD = -1.0
MASK_EPS = 1e-5    # host activity-test margin (superset of device mask)
MNEG = -30.0       # masked-score offset: exp(-30+L) ~ 1e-12
PAD_NODE = KP - 1  # all-padding node, used for unused slot entries


def build(nslot, n_cores=N_CORES, fake_ag=False, use_prelu=True):
    nc = bacc.Bacc("TRN2", target_bir_lowering=False, debug=False,
                   num_devices=n_cores)
    NS = nslot
    NH = NS - OWN_SLOTS          # halo slots (gathered in layer 2)
    assert NH >= 1

    # ---- kernel I/O (identical program on every core) ----
    # pts_sel5 rows: [p(3); sq; ones] for the slot sources
    # pts_own5 rows: [-2p(3); ones; sq] for the own destination columns
    pts_sel5_d = nc.dram_tensor("pts_sel5", [5, NS * 128], F32,
                                kind="ExternalInput")
    pts_own5_d = nc.dram_tensor("pts_own5", [5, IC], F32,
                                kind="ExternalInput")
    pts_own3_d = nc.dram_tensor("pts_own3", [3, IC], F32,
                                kind="ExternalInput")
    agidx_d = nc.dram_tensor("agidx", [128, NH], I32, kind="ExternalInput")
    # w1p/w2p: [W | W @ a_src_blockdiag] so es rides along with h
    w1p_d = nc.dram_tensor("w1p", [3, HCE], F32, kind="ExternalInput")
    w2p_d = nc.dram_tensor("w2p", [HC, HCE], BF16, kind="ExternalInput")
    adm1_d = nc.dram_tensor("adm1", [HC, H], BF16, kind="ExternalInput")
    adm2_d = nc.dram_tensor("adm2", [HC, H], BF16, kind="ExternalInput")
    bias1_d = nc.dram_tensor("bias1", [128, HC], F32, kind="ExternalInput")
    bias2_d = nc.dram_tensor("bias2", [128, HC], F32, kind="ExternalInput")
    fcw_d = nc.dram_tensor("fcw", [128, 2 * HC], F32, kind="ExternalInput")
    fcb_d = nc.dram_tensor("fcb", [128, 2], F32, kind="ExternalInput")
    ident_d = nc.dram_tensor("ident", [128, 128], F32, kind="ExternalInput")

    out_d = nc.dram_tensor("out", [IC, 2], F32, kind="ExternalOutput")

    with tile.TileContext(nc) as tc, ExitStack() as st:
        dram = st.enter_context(tc.tile_pool(name="dram", bufs=1,
                                             space="DRAM"))
        x_bounce = dram.tile([IC, HC], BF16)
        ag_out = dram.tile([KP, HC], BF16,
                           addr_space=("Local" if fake_ag else "Shared"))

        const = st.enter_context(tc.tile_pool(name="const", bufs=1))
        pts_sel5_sb = const.tile([5, NS * 128], F32)
        pts_own5_sb = const.tile([5, IC], F32)
        pts_own3_sb = const.tile([3, IC], F32)
        agidx_sb = const.tile([128, NH], I32)
        w1p_sb = const.tile([3, HCE], F32)
        w2p_sb = const.tile([128, 2, HCE], BF16)
        adm1_sb = const.tile([128, 2, H], BF16)
        adm2_sb = const.tile([128, 2, H], BF16)
        bias1_sb = const.tile([128, HC], F32)
        bias2_sb = const.tile([128, HC], F32)
        fcw_sb = const.tile([128, 2 * HC], F32)
        fcb_sb = const.tile([128, 2], F32)
        ident_sb = const.tile([128, 128], F32)
        ident_bf = const.tile([128, 128], BF16)

        nc.sync.dma_start(out=pts_sel5_sb[:, :], in_=pts_sel5_d[:, :])
        nc.sync.dma_start(out=pts_own5_sb[:, :], in_=pts_own5_d[:, :])
        nc.sync.dma_start(out=pts_own3_sb[:, :], in_=pts_own3_d[:, :])
        nc.sync.dma_start(out=agidx_sb[:, :], in_=agidx_d[:, :])
        nc.sync.dma_start(out=w1p_sb[:, :], in_=w1p_d[:, :])
        nc.sync.dma_start(out=w2p_sb[:, :, :],
                          in_=w2p_d.rearrange("(s p) c -> p s c", p=128))
        nc.sync.dma_start(out=adm1_sb[:, :, :],
                          in_=adm1_d.rearrange("(s p) h -> p s h", p=128))
        nc.sync.dma_start(out=adm2_sb[:, :, :],
                          in_=adm2_d.rearrange("(s p) h -> p s h", p=128))
        nc.sync.dma_start(out=bias1_sb[:, :], in_=bias1_d[:, :])
        nc.sync.dma_start(out=bias2_sb[:, :], in_=bias2_d[:, :])
        nc.sync.dma_start(out=fcw_sb[:, :], in_=fcw_d[:, :])
        nc.sync.dma_start(out=fcb_sb[:, :], in_=fcb_d[:, :])
        nc.sync.dma_start(out=ident_sb[:, :], in_=ident_d[:, :])
        nc.scalar.copy(ident_bf[:, :], ident_sb[:, :])

        big = st.enter_context(tc.tile_pool(name="big", bufs=1))
        hp_hi = big.tile([128, NS, H, CP1], BF16)
        hp_lo = big.tile([128, NS, H, CP1], BF16)
        es4 = big.tile([128, NS, H], F32)
        ed_b = big.tile([128, H, IC], BF16)
        x_sb = big.tile([128, ICT, HC], F32)
        x_bf = big.tile([128, ICT, HC], BF16)
        xt_own = big.tile([128, 2, IC], BF16)
        edt_sb = big.tile([H, IC], BF16)
        edt_rows = big.tile([1, H, IC], BF16)
        logit_sb = big.tile([128, ICT, 2], F32)
        mn_sb = big.tile([128, NS, IC], BF16)   # per-slot mask offsets

        nc.vector.memset(hp_hi[:, :, :, C:CP1], 1.0)
        nc.vector.memset(hp_lo[:, :, :, C:CP1], 0.0)

        # ================= the two GAT layers =================
        for layer in (1, 2):
            adm_sb = adm1_sb if layer == 1 else adm2_sb
            bias_sb = bias1_sb if layer == 1 else bias2_sb

            # ---- own-column side: hT(own), edT, ED broadcasts ----
            with tc.tile_pool(name=f"prep{layer}", bufs=2) as prep, \
                 tc.tile_pool(name=f"prep_ps{layer}", bufs=1,
                              space="PSUM") as prep_ps:
                ht_own = prep.tile([128, 2, IC], BF16, tag="ht", bufs=1)
                for oc in range(2):
                    ht_ps = prep_ps.tile([128, IC], F32, tag="ht_ps", bufs=1,
                                         name=f"ht_ps_{layer}_{oc}")
                    if layer == 1:
                        for lo, sz in ((0, 512), (512, 256)):
                            sl = slice(lo, lo + sz)
                            nc.tensor.matmul(
                                ht_ps[:, sl],
                                w1p_sb[:, oc * 128:(oc + 1) * 128],
                                pts_own3_sb[:, sl], start=True, stop=True)
                    else:
                        for s in range(2):
                            for lo, sz in ((0, 512), (512, 256)):
                                sl = slice(lo, lo + sz)
                                nc.tensor.matmul(
                                    ht_ps[:, sl],
                                    w2p_sb[:, s, oc * 128:(oc + 1) * 128],
                                    xt_own[:, s, sl],
                                    start=(s == 0), stop=(s == 1))
                    nc.scalar.copy(ht_own[:, oc, :], ht_ps[:, :])

                edt_ps = prep_ps.tile([H, IC], F32, tag="edt", bufs=1)
                for s in range(2):
                    for lo, sz in ((0, 512), (512, 256)):
                        sl = slice(lo, lo + sz)
                        nc.tensor.matmul(edt_ps[:, sl], adm_sb[:, s, :],
                                         ht_own[:, s, sl],
                                         start=(s == 0), stop=(s == 1))
                nc.scalar.copy(edt_sb[:, :], edt_ps[:, :])
                for h in range(H):
                    nc.sync.dma_start(out=edt_rows[0:1, h, :],
                                      in_=edt_sb[h:h + 1, :])
                for h in range(H):
                    nc.gpsimd.partition_broadcast(ed_b[:, h, :],
                                                  edt_rows[0:1, h, :])

            # ---- slot loop: h+es, d2 mask, scores, aggregation ----
            with tc.tile_pool(name=f"agg_ps{layer}", bufs=1,
                              space="PSUM") as agg_ps:
                aggp = [agg_ps.tile([128, 2, H, C], F32, tag=f"agg{p}",
                                    name=f"agg_{layer}_{p}")
                        for p in range(ICT // 2)]
                den_ps = agg_ps.tile([128, ICT, H], F32, tag="den",
                                     name=f"den_{layer}")
                with tc.tile_pool(name=f"jl{layer}", bufs=3) as jl, \
                     tc.tile_pool(name=f"h_ps{layer}", bufs=2,
                                  space="PSUM") as h_psp:
                    for s in range(NS):
                        # --- h + es for this slot's 128 sources ---
                        h_ps = h_psp.tile([128, HCE], F32, tag="h",
                                          name=f"h_ps_{layer}_{s}")
                        if layer == 1:
                            nc.tensor.matmul(
                                h_ps[:, :],
                                pts_sel5_sb[0:3, s * 128:(s + 1) * 128],
                                w1p_sb[:, :], start=True, stop=True)
                        elif s < OWN_SLOTS:
                            for half in range(2):
                                nc.tensor.matmul(
                                    h_ps[:, :],
                                    xt_own[:, half,
                                           s * 128:(s + 1) * 128],
                                    w2p_sb[:, half, :],
                                    start=(half == 0), stop=(half == 1))
                        else:
                            xg = jl.tile([128, HC], BF16, tag="xg",
                                         name=f"xg_{layer}_{s}")
                            nc.gpsimd.indirect_dma_start(
                                out=xg[:, :], out_offset=None,
                                in_=ag_out,
                                in_offset=bass.IndirectOffsetOnAxis(
                                    ap=agidx_sb[:, s - OWN_SLOTS:
                                                s - OWN_SLOTS + 1],
                                    axis=0))
                            xtg = jl.tile([128, 2, 128], BF16, tag="xtg",
                                          name=f"xtg_{layer}_{s}")
                            for half in range(2):
                                t_ps = h_psp.tile(
                                    [128, 128], BF16, tag=f"tr{half}",
                                    bufs=1, name=f"tr_{layer}_{s}_{half}")
                                nc.tensor.transpose(
                                    t_ps[:, :],
                                    xg[:, half * 128:(half + 1) * 128],
                                    ident_bf[:, :])
                                nc.scalar.copy(xtg[:, half, :], t_ps[:, :])
                            for half in range(2):
                                nc.tensor.matmul(
                                    h_ps[:, :], xtg[:, half, :],
                                    w2p_sb[:, half, :],
                                    start=(half == 0), stop=(half == 1))
                        nc.vector.tensor_scalar_add(es4[:, s, :],
                                                    h_ps[:, HC:HCE], 0.0)
                        # h -> bf16 hi + lo with ones/zeros column
                        nc.scalar.copy(
                            hp_hi[:, s, :, 0:C],
                            h_ps[:, 0:HC].rearrange("p (h c) -> p h c", h=H))
                        nc.vector.tensor_tensor(
                            hp_lo[:, s, :, 0:C],
                            h_ps[:, 0:HC].rearrange("p (h c) -> p h c", h=H),
                            hp_hi[:, s, :, 0:C], OP.subtract)

                        # --- mask offsets mn (layer 1: d2 on PE) ---
                        if layer == 1:
                            for lo, sz in ((0, 512), (512, 256)):
                                sl = slice(lo, lo + sz)
                                g_ps = h_psp.tile([128, sz], F32,
                                                  tag=f"g{lo}", bufs=1,
                                                  name=f"g_{s}_{lo}")
                                nc.tensor.matmul(
                                    g_ps[:, :],
                                    pts_sel5_sb[:, s * 128:(s + 1) * 128],
                                    pts_own5_sb[:, sl],
                                    start=True, stop=True)
                                nc.vector.tensor_scalar(
                                    mn_sb[:, s, sl], g_ps[:, :], R2, MNEG,
                                    OP.is_ge, OP.mult)

                        # --- scores: L = leaky(ed+es) + mn ; A = exp(L) ---
                        L4 = jl.tile([128, H, IC], BF16, tag="L4",
                                     name=f"L4_{layer}_{s}")
                        nc.scalar.activation(L4[:, 0, :], ed_b[:, 0, :],
                                             AF.Prelu,
                                             bias=es4[:, s, 0:1],
                                             scale=1.0, alpha=0.2)
                        for h in range(1, H):
                            nc.vector.tensor_scalar(
                                L4[:, h, :], ed_b[:, h, :],
                                es4[:, s, h:h + 1], None, OP.add)
                        nc.vector.scalar_tensor_tensor(
                            L4[:, 1:4, :], L4[:, 1:4, :], 0.2,
                            L4[:, 1:4, :], OP.mult, OP.max)
                        l4b, mnb = bass.broadcast_tensor_aps(
                            L4[:, :, :],
                            mn_sb[:, s:s + 1, :])
                        nc.vector.tensor_tensor(L4[:, :, :], l4b, mnb,
                                                OP.add)
                        A4 = jl.tile([128, H, IC], BF16, tag="A4",
                                     name=f"A4_{layer}_{s}")
                        nc.scalar.activation(A4[:, :, :], L4[:, :, :], AF.Exp)

                        # --- aggregation: hi+lo into one psum group per
                        # bank; two ic-chunks share each bank; denominators
                        # (ones column) accumulate in their own bank ---
                        for h in range(H):
                            for ic in range(ICT):
                                out_ap = aggp[ic // 2][:, ic % 2, h, :]
                                first = (s == 0 and h == 0 and ic % 2 == 0)
                                last = (s == NS - 1 and h == H - 1
                                        and ic % 2 == 1)
                                nc.tensor.matmul(
                                    out_ap,
                                    A4[:, h, ic * 128:(ic + 1) * 128],
                                    hp_hi[:, s, h, 0:C],
                                    start=first, stop=False)
                                nc.tensor.matmul(
                                    out_ap,
                                    A4[:, h, ic * 128:(ic + 1) * 128],
                                    hp_lo[:, s, h, 0:C],
                                    start=False, stop=last)
                                nc.tensor.matmul(
                                    den_ps[:, ic, h:h + 1],
                                    A4[:, h, ic * 128:(ic + 1) * 128],
                                    hp_hi[:, s, h, C:CP1],
                                    start=(s == 0 and h == 0 and ic == 0),
                                    stop=(s == NS - 1 and h == H - 1
                                          and ic == ICT - 1))

                # ---- finalize x = relu(num/den + b); AG or fc ----
                with tc.tile_pool(name=f"fin{layer}", bufs=2) as fin, \
                     tc.tile_pool(name=f"fin_ps{layer}", bufs=2,
                                  space="PSUM") as fin_ps:
                    for ic in range(ICT):
                        rec = fin.tile([128, H], F32, tag="rec",
                                       name=f"rec_{layer}_{ic}")
                        nc.vector.reciprocal(rec[:, :], den_ps[:, ic, :])
                        for h in range(H):
                            nc.vector.scalar_tensor_tensor(
                                x_sb[:, ic, h * C:(h + 1) * C],
                                aggp[ic // 2][:, ic % 2, h, :],
                                rec[:, h:h + 1],
                                bias_sb[:, h * C:(h + 1) * C],
                                OP.mult, OP.add)
                    nc.vector.tensor_scalar(x_sb[:, :, :], x_sb[:, :, :],
                                            0.0, None, OP.max)

                    if layer == 1:
                        # node-major payload: AG starts as soon as x is done
                        nc.scalar.copy(x_bf[:, :, :], x_sb[:, :, :])
                        nc.sync.dma_start(
                            out=x_bounce.rearrange("(q p) c -> p q c",
                                                   p=128),
                            in_=x_bf[:, :, :])
                        if fake_ag:
                            for r in range(n_cores):
                                nc.sync.dma_start(
                                    out=ag_out[r * IC:(r + 1) * IC, :],
                                    in_=x_bounce[:, :])
                        else:
                            nc.gpsimd.collective_compute(
                                "AllGather", OP.bypass,
                                replica_groups=[list(range(n_cores))],
                                ins=[x_bounce.opt()],
                                outs=[ag_out.opt()])
                        # transposed own x, needed for layer-2 prep
                        for ic in range(ICT):
                            for oc in range(2):
                                t_ps = fin_ps.tile([128, 128], F32,
                                                   tag="t_ps",
                                                   name=f"t_ps_{ic}_{oc}")
                                nc.tensor.transpose(
                                    t_ps[:, :],
                                    x_sb[:, ic, oc * 128:(oc + 1) * 128],
                                    ident_sb[:, :])
                                nc.scalar.copy(
                                    xt_own[:, oc, ic * 128:(ic + 1) * 128],
                                    t_ps[:, :])
                    else:
                        for ic in range(ICT):
                            for o in range(2):
                                prod = fin.tile([128, HC], F32, tag="prod",
                                                name=f"prod_{ic}_{o}")
                                nc.vector.tensor_tensor(
                                    prod[:, :], x_sb[:, ic, :],
                                    fcw_sb[:, o * HC:(o + 1) * HC], OP.mult)
                                red = fin.tile([128, 1], F32, tag="red",
                                               name=f"red_{ic}_{o}")
                                nc.vector.tensor_reduce(
                                    red[:, :], prod[:, :], AX.X, OP.add)
                                nc.vector.tensor_scalar_add(
                                    logit_sb[:, ic, o:o + 1], red[:, :],
                                    fcb_sb[:, o:o + 1])
                        nc.sync.dma_start(
                            out=out_d.rearrange("(q p) o -> p q o", p=128),
                            in_=logit_sb[:, :, :])

    nc.compile()
    return nc


_BUILD_CACHE = {}


def _get_nc(nslot, use_prelu=True):
    key = (nslot, use_prelu)
    if key not in _BUILD_CACHE:
        _BUILD_CACHE[key] = build(nslot, use_prelu=use_prelu)
    return _BUILD_CACHE[key]


def _morton(p, bits=10):
    q = np.clip((p * (1 << bits)).astype(np.int64), 0, (1 << bits) - 1)
    code = np.zeros(len(p), np.int64)
    for b in range(bits):
        for dim in range(3):
            code |= ((q[:, dim] >> b) & 1) << (3 * b + dim)
    return code


def _plan(pts):
    """Sort nodes spatially, build each core's compacted source list."""
    order = np.argsort(_morton(pts), kind="stable")
    p_sorted = np.full((KP, 3), PAD_COORD, np.float32)
    p_sorted[:K] = pts[order]

    sq = (p_sorted ** 2).sum(-1, dtype=np.float32)
    G = p_sorted @ p_sorted.T
    d2 = sq[None, :] + sq[:, None] - 2.0 * G
    near = d2 < (R2 + MASK_EPS)          # [j, i], conservative superset

    lists = []
    for c in range(N_CORES):
        act = np.flatnonzero(near[:, c * IC:(c + 1) * IC].any(axis=1))
        own = np.arange(c * IC, (c + 1) * IC)
        halo = act[(act < c * IC) | (act >= (c + 1) * IC)]
        lists.append(np.concatenate([own, halo]))
    nslot = max(OWN_SLOTS + 1,
                max((len(l) + 127) // 128 for l in lists))
    lists = [np.concatenate(
        [l, np.full(nslot * 128 - len(l), PAD_NODE, l.dtype)])
        for l in lists]
    return order, p_sorted, lists, nslot


def _prep_inputs(pos, pos_non_manifold, W1, a_src1, a_dst1, b1,
                 W2, a_src2, a_dst2, b2, fc_w, fc_b):
    bf16 = ml_dtypes.bfloat16
    pts = np.concatenate([np.asarray(pos, np.float32),
                          np.asarray(pos_non_manifold, np.float32)],
                         axis=2)[0].T  # [K, 3]
    order, p_sorted, lists, nslot = _plan(pts)
    sq_sorted = (p_sorted ** 2).sum(-1, dtype=np.float32).astype(np.float32)

    def bcast128(v):
        v = np.asarray(v, np.float32).reshape(-1)
        return np.ascontiguousarray(
            np.broadcast_to(v[None, :], (128, v.size)))

    def blockdiag(a):  # [H, C] -> [HC, H] fp32
        m = np.zeros((HC, H), dtype=np.float32)
        for h in range(H):
            m[h * C:(h + 1) * C, h] = np.asarray(a, np.float32)[h]
        return m

    W1f = np.asarray(W1, np.float32)
    W2f = np.asarray(W2, np.float32)
    w1p = np.concatenate([W1f, W1f @ blockdiag(a_src1)], axis=1)
    w2p = np.concatenate([W2f, W2f @ blockdiag(a_src2)], axis=1)

    shared = {
        "w1p": np.ascontiguousarray(w1p.astype(np.float32)),
        "w2p": np.ascontiguousarray(w2p.astype(bf16)),
        "adm1": blockdiag(a_dst1).astype(bf16),
        "adm2": blockdiag(a_dst2).astype(bf16),
        "bias1": bcast128(b1),
        "bias2": bcast128(b2),
        "fcw": bcast128(np.asarray(fc_w, np.float32).T),
        "fcb": bcast128(fc_b),
        "ident": np.eye(128, dtype=np.float32),
    }
    in_maps = []
    for c in range(N_CORES):
        sel = lists[c]
        psel = p_sorted[sel]                      # [nslot*128, 3]
        pown = p_sorted[c * IC:(c + 1) * IC]
        sel5 = np.concatenate(
            [psel.T, sq_sorted[sel][None, :],
             np.ones((1, len(sel)), np.float32)], axis=0)
        own5 = np.concatenate(
            [-2.0 * pown.T, np.ones((1, IC), np.float32),
             (pown ** 2).sum(-1, dtype=np.float32)[None, :]], axis=0)
        # halo slots index node-major ag rows directly
        agidx = np.ascontiguousarray(
            sel[OWN_SLOTS * 128:].reshape(-1, 128).T.astype(np.int32))
        m = dict(shared)
        m["pts_sel5"] = np.ascontiguousarray(sel5.astype(np.float32))
        m["pts_own5"] = np.ascontiguousarray(own5.astype(np.float32))
        m["pts_own3"] = np.ascontiguousarray(pown.T)
        m["agidx"] = agidx
        in_maps.append(m)
    return in_maps, order, nslot


def kernel(pos, pos_non_manifold, W1, a_src1, a_dst1, b1,
           W2, a_src2, a_dst2, b2, fc_w, fc_b, _trace=False,
           _use_prelu=True):
    in_maps, order, nslot = _prep_inputs(
        pos, pos_non_manifold, W1, a_src1, a_dst1, b1,
        W2, a_src2, a_dst2, b2, fc_w, fc_b)
    nc = _get_nc(nslot, use_prelu=_use_prelu)
    res = run_bass_kernel_spmd(nc, in_maps, core_ids=list(range(N_CORES)),
                               trace=_trace)
    kernel.last_results = res
    x2s = np.concatenate([res.results[c]["out"] for c in range(N_CORES)],
                         axis=0)  # [KP, 2] in sorted order
    x2 = np.empty((K, 2), np.float32)
    x2[order] = x2s[:K]
    logits = np.ascontiguousarray(x2[M:K]).reshape(1, 2, 3000)
    return logits.astype(np.float32)


# revision 18
# speedup vs baseline: 3.2181x; 1.1642x over previous
"""Trainium2 Bass kernel for a 2-layer GAT occupancy predictor (B=1).

Reference math:
  pts = concat(pos, pos_non_manifold) -> [K=6000, 3]
  mask[i,j] = ||pts_i - pts_j||^2 < 0.05^2          (dense radius graph)
  layer l:  h = x @ Wl                              [K, 4*64]
            e[i,j,h] = leaky02(ed[i,h] + es[j,h])   es/ed = <h, a_src/dst>
            alpha = softmax_j(e masked)
            x' = relu(alpha @ h + b)
  logits = (x2 @ fc_w + fc_b)[M:] reshaped to [1, 2, 3000]

Distribution (8 NeuronCores): nodes are Morton-sorted on the host so the
radius graph becomes block-local; core c owns destination rows
[768c, 768(c+1)) of the sorted, padded 6144-node graph.  The radius mask is
>99.9% empty, so each core only processes the ~870 source nodes that can
reach its destination block: the host gathers them into NSLOT=8 compacted
128-node slots ordered [own 768 | halo | pad], a conservative
epsilon-superset of the exact mask, padded to a common NSLOT so all cores
run the same program on different slot data.

Layer 1 is fully static: the host passes the gathered slot points.  Between
layers one AllGather shares the node-major features x [768, 256]; layer 2
computes own-slot h from the local transposed features and fetches the halo
slots' x rows from the gathered buffer with one indirect DMA per slot,
transposing on the PE.

Engine mapping per slot:
  PE   : h (with es riding along as 4 extra host-folded weight columns),
         d2 = |p_i - p_j|^2 as one K=5 matmul ([p; sq; 1] x [-2p; 1; sq]),
         the alpha @ h aggregation as bf16 hi+lo pairs into shared PSUM
         banks, denominators as N=1 ones-column matmuls.
  DVE  : scores e = ed+es, T = 0.2e, leaky = max merged over
         heads (2x mode), part of the mask-adds, h hi/lo split.
  ACT  : one head's leaky via Prelu, exp over all 4 heads in one op,
         PSUM->SBUF copies.
  Pool : the other mask-adds, partition-broadcast of ed.
Mask offsets (-30/0 bf16) stay resident in SBUF across both layers.
Padded nodes sit at (-1,-1,-1): finite features, outside every real radius.
"""

import sys

sys.path.insert(0, "/opt/trn_rl_repo")

from contextlib import ExitStack

import ml_dtypes
import numpy as np

import concourse.bacc as bacc
import concourse.bass as bass
import concourse.mybir as mybir
import concourse.tile as tile
from concourse.bass_utils import run_bass_kernel_spmd

F32 = mybir.dt.float32
BF16 = mybir.dt.bfloat16
I32 = mybir.dt.int32
AF = mybir.ActivationFunctionType
OP = mybir.AluOpType
AX = mybir.AxisListType

N_CORES = 8
N = 3000
M = 3000
K = N + M          # real nodes
KP = 6144          # padded nodes
NT = KP // 128     # 48
IC = KP // N_CORES # 768 destinations per core
ICT = IC // 128    # 6 destination chunks per core
OWN_SLOTS = ICT    # first 6 slots are the core's own nodes
H = 4              # heads
C = 64             # channels per head
HC = H * C         # 256
HCE = HC + H       # h columns + es columns
CP1 = C + 1        # head channels + ones column
R2 = float(np.float32(0.05) * np.float32(0.05))
PAD_COORD = -1.0
MASK_EPS = 1e-5    # host activity-test margin (superset of device mask)
MNEG = -30.0       # masked-score offset: exp(-30+L) ~ 1e-12
PAD_NODE = KP - 1  # all-padding node, used for unused slot entries


def build(nslot, n_cores=N_CORES, fake_ag=False, use_prelu=True):
    nc = bacc.Bacc("TRN2", target_bir_lowering=False, debug=False,
                   num_devices=n_cores)
    NS = nslot
    NH = NS - OWN_SLOTS          # halo slots (gathered in layer 2)
    assert NH >= 1

    # ---- kernel I/O (identical program on every core) ----
    # pts_sel5 rows: [p(3); sq; ones] for the slot sources
    # pts_own5 rows: [-2p(3); ones; sq] for the own destination columns
    pts_sel5_d = nc.dram_tensor("pts_sel5", [5, NS * 128], F32,
                                kind="ExternalInput")
    pts_own5_d = nc.dram_tensor("pts_own5", [5, IC], F32,
                                kind="ExternalInput")
    pts_own3_d = nc.dram_tensor("pts_own3", [3, IC], F32,
                                kind="ExternalInput")
    agidx_d = nc.dram_tensor("agidx", [128, NH], I32, kind="ExternalInput")
    # w1p/w2p: [W | W @ a_src_blockdiag] so es rides along with h
    w1p_d = nc.dram_tensor("w1p", [3, HCE], F32, kind="ExternalInput")
    w2p_d = nc.dram_tensor("w2p", [HC, HCE], BF16, kind="ExternalInput")
    adm1_d = nc.dram_tensor("adm1", [HC, H], BF16, kind="ExternalInput")
    adm2_d = nc.dram_tensor("adm2", [HC, H], BF16, kind="ExternalInput")
    bias1_d = nc.dram_tensor("bias1", [128, HC], F32, kind="ExternalInput")
    bias2_d = nc.dram_tensor("bias2", [128, HC], F32, kind="ExternalInput")
    fcw_d = nc.dram_tensor("fcw", [128, 2 * HC], F32, kind="ExternalInput")
    fcb_d = nc.dram_tensor("fcb", [128, 2], F32, kind="ExternalInput")
    ident_d = nc.dram_tensor("ident", [128, 128], F32, kind="ExternalInput")

    out_d = nc.dram_tensor("out", [IC, 2], F32, kind="ExternalOutput")

    with tile.TileContext(nc) as tc, ExitStack() as st:
        dram = st.enter_context(tc.tile_pool(name="dram", bufs=1,
                                             space="DRAM"))
        x_bounce = dram.tile([IC, HC], BF16)
        ag_out = dram.tile([KP, HC], BF16,
                           addr_space=("Local" if fake_ag else "Shared"))

        const = st.enter_context(tc.tile_pool(name="const", bufs=1))
        pts_sel5_sb = const.tile([5, NS * 128], F32)
        pts_own5_sb = const.tile([5, IC], F32)
        pts_own3_sb = const.tile([3, IC], F32)
        agidx_sb = const.tile([128, NH], I32)
        w1p_sb = const.tile([3, HCE], F32)
        w2p_sb = const.tile([128, 2, HCE], BF16)
        adm1_sb = const.tile([128, 2, H], BF16)
        adm2_sb = const.tile([128, 2, H], BF16)
        bias1_sb = const.tile([128, HC], F32)
        bias2_sb = const.tile([128, HC], F32)
        fcw_sb = const.tile([128, 2 * HC], F32)
        fcb_sb = const.tile([128, 2], F32)
        ident_sb = const.tile([128, 128], F32)
        ident_bf = const.tile([128, 128], BF16)

        nc.sync.dma_start(out=pts_sel5_sb[:, :], in_=pts_sel5_d[:, :])
        nc.sync.dma_start(out=pts_own5_sb[:, :], in_=pts_own5_d[:, :])
        nc.sync.dma_start(out=pts_own3_sb[:, :], in_=pts_own3_d[:, :])
        nc.sync.dma_start(out=agidx_sb[:, :], in_=agidx_d[:, :])
        nc.sync.dma_start(out=w1p_sb[:, :], in_=w1p_d[:, :])
        nc.sync.dma_start(out=w2p_sb[:, :, :],
                          in_=w2p_d.rearrange("(s p) c -> p s c", p=128))
        nc.sync.dma_start(out=adm1_sb[:, :, :],
                          in_=adm1_d.rearrange("(s p) h -> p s h", p=128))
        nc.sync.dma_start(out=adm2_sb[:, :, :],
                          in_=adm2_d.rearrange("(s p) h -> p s h", p=128))
        nc.sync.dma_start(out=bias1_sb[:, :], in_=bias1_d[:, :])
        nc.sync.dma_start(out=bias2_sb[:, :], in_=bias2_d[:, :])
        nc.sync.dma_start(out=fcw_sb[:, :], in_=fcw_d[:, :])
        nc.sync.dma_start(out=fcb_sb[:, :], in_=fcb_d[:, :])
        nc.sync.dma_start(out=ident_sb[:, :], in_=ident_d[:, :])
        nc.scalar.copy(ident_bf[:, :], ident_sb[:, :])

        big = st.enter_context(tc.tile_pool(name="big", bufs=1))
        hp_hi = big.tile([128, NS, H, CP1], BF16)
        hp_lo = big.tile([128, NS, H, CP1], BF16)
        es4 = big.tile([128, NS, H], F32)
        ed_b = big.tile([128, H, IC], BF16)
        x_sb = big.tile([128, ICT, HC], F32)
        x_bf = big.tile([128, ICT, HC], BF16)
        xt_own = big.tile([128, 2, IC], BF16)
        edt_sb = big.tile([H, IC], BF16)
        edt_rows = big.tile([1, H, IC], BF16)
        logit_sb = big.tile([128, ICT, 2], F32)
        mn_sb = big.tile([128, NS, IC], BF16)   # per-slot mask offsets

        nc.vector.memset(hp_hi[:, :, :, C:CP1], 1.0)
        nc.vector.memset(hp_lo[:, :, :, C:CP1], 0.0)

        # ================= the two GAT layers =================
        for layer in (1, 2):
            adm_sb = adm1_sb if layer == 1 else adm2_sb
            bias_sb = bias1_sb if layer == 1 else bias2_sb

            # ---- own-column side: hT(own), edT, ED broadcasts ----
            with tc.tile_pool(name=f"prep{layer}", bufs=2) as prep, \
                 tc.tile_pool(name=f"prep_ps{layer}", bufs=1,
                              space="PSUM") as prep_ps:
                ht_own = prep.tile([128, 2, IC], BF16, tag="ht", bufs=1)
                for oc in range(2):
                    ht_ps = prep_ps.tile([128, IC], F32, tag="ht_ps", bufs=1,
                                         name=f"ht_ps_{layer}_{oc}")
                    if layer == 1:
                        for lo, sz in ((0, 512), (512, 256)):
                            sl = slice(lo, lo + sz)
                            nc.tensor.matmul(
                                ht_ps[:, sl],
                                w1p_sb[:, oc * 128:(oc + 1) * 128],
                                pts_own3_sb[:, sl], start=True, stop=True)
                    else:
                        for s in range(2):
                            for lo, sz in ((0, 512), (512, 256)):
                                sl = slice(lo, lo + sz)
                                nc.tensor.matmul(
                                    ht_ps[:, sl],
                                    w2p_sb[:, s, oc * 128:(oc + 1) * 128],
                                    xt_own[:, s, sl],
                                    start=(s == 0), stop=(s == 1))
                    nc.scalar.copy(ht_own[:, oc, :], ht_ps[:, :])

                edt_ps = prep_ps.tile([H, IC], F32, tag="edt", bufs=1)
                for s in range(2):
                    for lo, sz in ((0, 512), (512, 256)):
                        sl = slice(lo, lo + sz)
                        nc.tensor.matmul(edt_ps[:, sl], adm_sb[:, s, :],
                                         ht_own[:, s, sl],
                                         start=(s == 0), stop=(s == 1))
                nc.scalar.copy(edt_sb[:, :], edt_ps[:, :])
                for h in range(H):
                    nc.sync.dma_start(out=edt_rows[0:1, h, :],
                                      in_=edt_sb[h:h + 1, :])
                for h in range(H):
                    nc.gpsimd.partition_broadcast(ed_b[:, h, :],
                                                  edt_rows[0:1, h, :])

            # ---- slot loop: h+es, d2 mask, scores, aggregation ----
            with tc.tile_pool(name=f"agg_ps{layer}", bufs=1,
                              space="PSUM") as agg_ps:
                aggp = [agg_ps.tile([128, 2, H, C], F32, tag=f"agg{p}",
                                    name=f"agg_{layer}_{p}")
                        for p in range(ICT // 2)]
                den_ps = agg_ps.tile([128, ICT, H], F32, tag="den",
                                     name=f"den_{layer}")
                with tc.tile_pool(name=f"jl{layer}", bufs=3) as jl, \
                     tc.tile_pool(name=f"h_ps{layer}", bufs=2,
                                  space="PSUM") as h_psp:
                    for s in range(NS):
                        # halo slots wait on the AllGather: tell the static
                        # scheduler to order their chain after the own slots
                        # so in-order engine queues don't stall behind it
                        if layer == 2 and s >= OWN_SLOTS:
                            tc.tile_set_cur_wait(0.5)
                        # --- h + es for this slot's 128 sources ---
                        h_ps = h_psp.tile([128, HCE], F32, tag="h",
                                          name=f"h_ps_{layer}_{s}")
                        if layer == 1:
                            nc.tensor.matmul(
                                h_ps[:, :],
                                pts_sel5_sb[0:3, s * 128:(s + 1) * 128],
                                w1p_sb[:, :], start=True, stop=True)
                        elif s < OWN_SLOTS:
                            for half in range(2):
                                nc.tensor.matmul(
                                    h_ps[:, :],
                                    xt_own[:, half,
                                           s * 128:(s + 1) * 128],
                                    w2p_sb[:, half, :],
                                    start=(half == 0), stop=(half == 1))
                        else:
                            xg = jl.tile([128, HC], BF16, tag="xg",
                                         name=f"xg_{layer}_{s}")
                            nc.gpsimd.indirect_dma_start(
                                out=xg[:, :], out_offset=None,
                                in_=ag_out,
                                in_offset=bass.IndirectOffsetOnAxis(
                                    ap=agidx_sb[:, s - OWN_SLOTS:
                                                s - OWN_SLOTS + 1],
                                    axis=0))
                            xtg = jl.tile([128, 2, 128], BF16, tag="xtg",
                                          name=f"xtg_{layer}_{s}")
                            for half in range(2):
                                t_ps = h_psp.tile(
                                    [128, 128], BF16, tag=f"tr{half}",
                                    bufs=1, name=f"tr_{layer}_{s}_{half}")
                                nc.tensor.transpose(
                                    t_ps[:, :],
                                    xg[:, half * 128:(half + 1) * 128],
                                    ident_bf[:, :])
                                nc.scalar.copy(xtg[:, half, :], t_ps[:, :])
                            for half in range(2):
                                nc.tensor.matmul(
                                    h_ps[:, :], xtg[:, half, :],
                                    w2p_sb[:, half, :],
                                    start=(half == 0), stop=(half == 1))
                        nc.vector.tensor_scalar_add(es4[:, s, :],
                                                    h_ps[:, HC:HCE], 0.0)
                        # h -> bf16 hi + lo with ones/zeros column
                        nc.scalar.copy(
                            hp_hi[:, s, :, 0:C],
                            h_ps[:, 0:HC].rearrange("p (h c) -> p h c", h=H))
                        nc.vector.tensor_tensor(
                            hp_lo[:, s, :, 0:C],
                            h_ps[:, 0:HC].rearrange("p (h c) -> p h c", h=H),
                            hp_hi[:, s, :, 0:C], OP.subtract)

                        # --- mask offsets mn (layer 1: d2 on PE) ---
                        if layer == 1:
                            for lo, sz in ((0, 512), (512, 256)):
                                sl = slice(lo, lo + sz)
                                g_ps = h_psp.tile([128, sz], F32,
                                                  tag=f"g{lo}", bufs=1,
                                                  name=f"g_{s}_{lo}")
                                nc.tensor.matmul(
                                    g_ps[:, :],
                                    pts_sel5_sb[:, s * 128:(s + 1) * 128],
                                    pts_own5_sb[:, sl],
                                    start=True, stop=True)
                                nc.vector.tensor_scalar(
                                    mn_sb[:, s, sl], g_ps[:, :], R2, MNEG,
                                    OP.is_ge, OP.mult)

                        # --- scores: L = leaky(ed+es) + mn ; A = exp(L) ---
                        L4 = jl.tile([128, H, IC], BF16, tag="L4",
                                     name=f"L4_{layer}_{s}")
                        nc.scalar.activation(L4[:, 0, :], ed_b[:, 0, :],
                                             AF.Prelu,
                                             bias=es4[:, s, 0:1],
                                             scale=1.0, alpha=0.2)
                        T4 = jl.tile([128, 3, IC], BF16, tag="T4",
                                     name=f"T4_{layer}_{s}")
                        for h in range(1, H):
                            nc.vector.tensor_scalar(
                                L4[:, h, :], ed_b[:, h, :],
                                es4[:, s, h:h + 1], None, OP.add)
                        nc.vector.tensor_scalar(
                            T4[:, :, :], L4[:, 1:4, :], 0.2, None, OP.mult)
                        nc.vector.tensor_tensor(L4[:, 1:4, :], L4[:, 1:4, :],
                                                T4[:, :, :], OP.max)
                        l4b, mnb = bass.broadcast_tensor_aps(
                            L4[:, :, :],
                            mn_sb[:, s:s + 1, :])
                        nc.vector.tensor_tensor(L4[:, :, :], l4b, mnb,
                                                OP.add)
                        A4 = jl.tile([128, H, IC], BF16, tag="A4",
                                     name=f"A4_{layer}_{s}")
                        nc.scalar.activation(A4[:, :, :], L4[:, :, :], AF.Exp)

                        # --- aggregation: hi+lo into one psum group per
                        # bank; two ic-chunks share each bank; denominators
                        # (ones column) accumulate in their own bank ---
                        for h in range(H):
                            for ic in range(ICT):
                                out_ap = aggp[ic // 2][:, ic % 2, h, :]
                                first = (s == 0 and h == 0 and ic % 2 == 0)
                                last = (s == NS - 1 and h == H - 1
                                        and ic % 2 == 1)
                                nc.tensor.matmul(
                                    out_ap,
                                    A4[:, h, ic * 128:(ic + 1) * 128],
                                    hp_hi[:, s, h, 0:C],
                                    start=first, stop=False)
                                nc.tensor.matmul(
                                    out_ap,
                                    A4[:, h, ic * 128:(ic + 1) * 128],
                                    hp_lo[:, s, h, 0:C],
                                    start=False, stop=last)
                                nc.tensor.matmul(
                                    den_ps[:, ic, h:h + 1],
                                    A4[:, h, ic * 128:(ic + 1) * 128],
                                    hp_hi[:, s, h, C:CP1],
                                    start=(s == 0 and h == 0 and ic == 0),
                                    stop=(s == NS - 1 and h == H - 1
                                          and ic == ICT - 1))
                    tc.cur_wait_ts = None

                # ---- finalize x = relu(num/den + b); AG or fc ----
                with tc.tile_pool(name=f"fin{layer}", bufs=2) as fin, \
                     tc.tile_pool(name=f"fin_ps{layer}", bufs=2,
                                  space="PSUM") as fin_ps:
                    for ic in range(ICT):
                        rec = fin.tile([128, H], F32, tag="rec",
                                       name=f"rec_{layer}_{ic}")
                        nc.vector.reciprocal(rec[:, :], den_ps[:, ic, :])
                        for h in range(H):
                            nc.vector.scalar_tensor_tensor(
                                x_sb[:, ic, h * C:(h + 1) * C],
                                aggp[ic // 2][:, ic % 2, h, :],
                                rec[:, h:h + 1],
                                bias_sb[:, h * C:(h + 1) * C],
                                OP.mult, OP.add)
                    nc.vector.tensor_scalar(x_sb[:, :, :], x_sb[:, :, :],
                                            0.0, None, OP.max)

                    if layer == 1:
                        # node-major payload: AG starts as soon as x is done
                        nc.scalar.copy(x_bf[:, :, :], x_sb[:, :, :])
                        nc.sync.dma_start(
                            out=x_bounce.rearrange("(q p) c -> p q c",
                                                   p=128),
                            in_=x_bf[:, :, :])
                        if fake_ag:
                            for r in range(n_cores):
                                nc.sync.dma_start(
                                    out=ag_out[r * IC:(r + 1) * IC, :],
                                    in_=x_bounce[:, :])
                        else:
                            nc.gpsimd.collective_compute(
                                "AllGather", OP.bypass,
                                replica_groups=[list(range(n_cores))],
                                ins=[x_bounce.opt()],
                                outs=[ag_out.opt()])
                        # transposed own x, needed for layer-2 prep
                        for ic in range(ICT):
                            for oc in range(2):
                                t_ps = fin_ps.tile([128, 128], F32,
                                                   tag="t_ps",
                                                   name=f"t_ps_{ic}_{oc}")
                                nc.tensor.transpose(
                                    t_ps[:, :],
                                    x_sb[:, ic, oc * 128:(oc + 1) * 128],
                                    ident_sb[:, :])
                                nc.scalar.copy(
                                    xt_own[:, oc, ic * 128:(ic + 1) * 128],
                                    t_ps[:, :])
                    else:
                        for ic in range(ICT):
                            for o in range(2):
                                prod = fin.tile([128, HC], F32, tag="prod",
                                                name=f"prod_{ic}_{o}")
                                nc.vector.tensor_tensor(
                                    prod[:, :], x_sb[:, ic, :],
                                    fcw_sb[:, o * HC:(o + 1) * HC], OP.mult)
                                red = fin.tile([128, 1], F32, tag="red",
                                               name=f"red_{ic}_{o}")
                                nc.vector.tensor_reduce(
                                    red[:, :], prod[:, :], AX.X, OP.add)
                                nc.vector.tensor_scalar_add(
                                    logit_sb[:, ic, o:o + 1], red[:, :],
                                    fcb_sb[:, o:o + 1])
                        nc.sync.dma_start(
                            out=out_d.rearrange("(q p) o -> p q o", p=128),
                            in_=logit_sb[:, :, :])

    nc.compile()
    return nc


_BUILD_CACHE = {}


def _get_nc(nslot, use_prelu=True):
    key = (nslot, use_prelu)
    if key not in _BUILD_CACHE:
        _BUILD_CACHE[key] = build(nslot, use_prelu=use_prelu)
    return _BUILD_CACHE[key]


def _morton(p, bits=10):
    q = np.clip((p * (1 << bits)).astype(np.int64), 0, (1 << bits) - 1)
    code = np.zeros(len(p), np.int64)
    for b in range(bits):
        for dim in range(3):
            code |= ((q[:, dim] >> b) & 1) << (3 * b + dim)
    return code


def _plan(pts):
    """Sort nodes spatially, build each core's compacted source list."""
    order = np.argsort(_morton(pts), kind="stable")
    p_sorted = np.full((KP, 3), PAD_COORD, np.float32)
    p_sorted[:K] = pts[order]

    sq = (p_sorted ** 2).sum(-1, dtype=np.float32)
    G = p_sorted @ p_sorted.T
    d2 = sq[None, :] + sq[:, None] - 2.0 * G
    near = d2 < (R2 + MASK_EPS)          # [j, i], conservative superset

    lists = []
    for c in range(N_CORES):
        act = np.flatnonzero(near[:, c * IC:(c + 1) * IC].any(axis=1))
        own = np.arange(c * IC, (c + 1) * IC)
        halo = act[(act < c * IC) | (act >= (c + 1) * IC)]
        lists.append(np.concatenate([own, halo]))
    nslot = max(OWN_SLOTS + 1,
                max((len(l) + 127) // 128 for l in lists))
    lists = [np.concatenate(
        [l, np.full(nslot * 128 - len(l), PAD_NODE, l.dtype)])
        for l in lists]
    return order, p_sorted, lists, nslot


def _prep_inputs(pos, pos_non_manifold, W1, a_src1, a_dst1, b1,
                 W2, a_src2, a_dst2, b2, fc_w, fc_b):
    bf16 = ml_dtypes.bfloat16
    pts = np.concatenate([np.asarray(pos, np.float32),
                          np.asarray(pos_non_manifold, np.float32)],
                         axis=2)[0].T  # [K, 3]
    order, p_sorted, lists, nslot = _plan(pts)
    sq_sorted = (p_sorted ** 2).sum(-1, dtype=np.float32).astype(np.float32)

    def bcast128(v):
        v = np.asarray(v, np.float32).reshape(-1)
        return np.ascontiguousarray(
            np.broadcast_to(v[None, :], (128, v.size)))

    def blockdiag(a):  # [H, C] -> [HC, H] fp32
        m = np.zeros((HC, H), dtype=np.float32)
        for h in range(H):
            m[h * C:(h + 1) * C, h] = np.asarray(a, np.float32)[h]
        return m

    W1f = np.asarray(W1, np.float32)
    W2f = np.asarray(W2, np.float32)
    w1p = np.concatenate([W1f, W1f @ blockdiag(a_src1)], axis=1)
    w2p = np.concatenate([W2f, W2f @ blockdiag(a_src2)], axis=1)

    shared = {
        "w1p": np.ascontiguousarray(w1p.astype(np.float32)),
        "w2p": np.ascontiguousarray(w2p.astype(bf16)),
        "adm1": blockdiag(a_dst1).astype(bf16),
        "adm2": blockdiag(a_dst2).astype(bf16),
        "bias1": bcast128(b1),
        "bias2": bcast128(b2),
        "fcw": bcast128(np.asarray(fc_w, np.float32).T),
        "fcb": bcast128(fc_b),
        "ident": np.eye(128, dtype=np.float32),
    }
    in_maps = []
    for c in range(N_CORES):
        sel = lists[c]
        psel = p_sorted[sel]                      # [nslot*128, 3]
        pown = p_sorted[c * IC:(c + 1) * IC]
        sel5 = np.concatenate(
            [psel.T, sq_sorted[sel][None, :],
             np.ones((1, len(sel)), np.float32)], axis=0)
        own5 = np.concatenate(
            [-2.0 * pown.T, np.ones((1, IC), np.float32),
             (pown ** 2).sum(-1, dtype=np.float32)[None, :]], axis=0)
        # halo slots index node-major ag rows directly
        agidx = np.ascontiguousarray(
            sel[OWN_SLOTS * 128:].reshape(-1, 128).T.astype(np.int32))
        m = dict(shared)
        m["pts_sel5"] = np.ascontiguousarray(sel5.astype(np.float32))
        m["pts_own5"] = np.ascontiguousarray(own5.astype(np.float32))
        m["pts_own3"] = np.ascontiguousarray(pown.T)
        m["agidx"] = agidx
        in_maps.append(m)
    return in_maps, order, nslot


def kernel(pos, pos_non_manifold, W1, a_src1, a_dst1, b1,
           W2, a_src2, a_dst2, b2, fc_w, fc_b, _trace=False,
           _use_prelu=True):
    in_maps, order, nslot = _prep_inputs(
        pos, pos_non_manifold, W1, a_src1, a_dst1, b1,
        W2, a_src2, a_dst2, b2, fc_w, fc_b)
    nc = _get_nc(nslot, use_prelu=_use_prelu)
    res = run_bass_kernel_spmd(nc, in_maps, core_ids=list(range(N_CORES)),
                               trace=_trace)
    kernel.last_results = res
    x2s = np.concatenate([res.results[c]["out"] for c in range(N_CORES)],
                         axis=0)  # [KP, 2] in sorted order
    x2 = np.empty((K, 2), np.float32)
    x2[order] = x2s[:K]
    logits = np.ascontiguousarray(x2[M:K]).reshape(1, 2, 3000)
    return logits.astype(np.float32)
